# revision 6
# baseline (speedup 1.0000x reference)
"""Trainium2 Bass kernel for an 8-expert top-2 MoE layer (768 hidden, 3072 FFN).

Strategy (expert-parallel over 8 NeuronCores):
  - Each core owns one expert's FFN weights (w1[e], b1[e], w2[e], b2[e]).
  - The host computes routing indices (which tokens go to which expert) and
    gathers/pads each expert's tokens to a fixed capacity C; the device
    computes gelu(x@w1+b1)@w2+b2 scaled by the gate for those tokens.
  - The router itself (logits, softmax, top-2 dispatch mask, aux-loss partial
    sums) is ALSO computed on device, data-parallel: core i handles tokens
    [i*T/8, (i+1)*T/8).
  - The host scatters the per-expert outputs back (each token receives
    exactly TOP_K=2 contributions) and combines aux-loss partials.

All heavy math runs on device. Matmuls use float32r (fp32 data at ~1
cycle/row on the PE when the moving free dim >= 256).
"""

import os
import numpy as np

P = 128
E = 8
H = 768
F = 3072
KH = H // P      # 6
KF = F // P      # 24
CHUNK = 256      # tokens per mm1 moving block (>=256 keeps float32r full-rate)
N_CORES = 8

_PROGRAM_CACHE = {}


def build_program(C, TS, reps=1, mm_dtype_name="float32r"):
    """Build + compile the per-core SPMD Bass program.

    C:  padded per-expert token capacity (multiple of CHUNK)
    TS: tokens per core for the router section (T / 8)
    reps: repeat the whole body (for wall-clock delta timing); outputs are
          rewritten identically each rep.
    """
    import concourse.mybir as mybir
    import concourse.tile as tile
    from concourse import bacc

    f32 = mybir.dt.float32
    mdt = getattr(mybir.dt, mm_dtype_name)
    AF = mybir.ActivationFunctionType
    OP = mybir.AluOpType
    X = mybir.AxisListType.X

    assert C % CHUNK == 0 and TS % P == 0
    nch = C // CHUNK
    ntr = TS // P

    nc = bacc.Bacc(None, target_bir_lowering=False)

    xT_ffn = nc.dram_tensor("xT_ffn", [P, KH, C], mdt, kind="ExternalInput")
    w1_d = nc.dram_tensor("w1", [P, KH, F], mdt, kind="ExternalInput")
    w2_d = nc.dram_tensor("w2", [P, KF, H], mdt, kind="ExternalInput")
    b1_d = nc.dram_tensor("b1", [P, KF], f32, kind="ExternalInput")
    b2_d = nc.dram_tensor("b2", [P, H], f32, kind="ExternalInput")
    g_d = nc.dram_tensor("gates", [P, C // P], f32, kind="ExternalInput")
    xr_d = nc.dram_tensor("x_r", [P, KH, TS], f32, kind="ExternalInput")
    rwt_d = nc.dram_tensor("rwght", [P, KH, E], f32, kind="ExternalInput")
    rb_d = nc.dram_tensor("rbias", [P, E], f32, kind="ExternalInput")

    y_d = nc.dram_tensor("y", [C, H], f32, kind="ExternalOutput")
    lg_d = nc.dram_tensor("logits", [TS, E], f32, kind="ExternalOutput")
    sm_d = nc.dram_tensor("softmax", [TS, E], f32, kind="ExternalOutput")
    mk_d = nc.dram_tensor("mask", [TS, E], f32, kind="ExternalOutput")
    s_d = nc.dram_tensor("sums", [1, 2 * E], f32, kind="ExternalOutput")

    with tile.TileContext(nc) as tc:
        with (
            tc.tile_pool(name="wpool", bufs=1) as wpool,
            tc.tile_pool(name="cpool", bufs=1) as cpool,
            tc.tile_pool(name="xpool", bufs=2) as xpool,
            tc.tile_pool(name="hpool", bufs=1) as hpool,
            tc.tile_pool(name="ypool", bufs=2) as ypool,
            tc.tile_pool(name="rpool", bufs=2) as rpool,
            tc.tile_pool(name="ps_h", bufs=2, space="PSUM") as ps_h,
            tc.tile_pool(name="ps_y", bufs=2, space="PSUM") as ps_y,
            tc.tile_pool(name="ps_r", bufs=1, space="PSUM") as ps_r,
        ):
            for _rep in range(reps):
                w1_sb = wpool.tile([P, KH, F], mdt, tag="w1")
                nc.sync.dma_start(w1_sb[:], w1_d[:])
                w2_sb = wpool.tile([P, KF, H], mdt, tag="w2")
                nc.sync.dma_start(w2_sb[:], w2_d[:])
                b1_sb = cpool.tile([P, KF], f32, tag="b1")
                nc.sync.dma_start(b1_sb[:], b1_d[:])
                b2_sb = cpool.tile([P, H], f32, tag="b2")
                nc.sync.dma_start(b2_sb[:], b2_d[:])
                g_sb = cpool.tile([P, C // P], f32, tag="g")
                nc.sync.dma_start(g_sb[:], g_d[:])
                rwt_sb = cpool.tile([P, KH, E], f32, tag="rwt")
                nc.sync.dma_start(rwt_sb[:], rwt_d[:])
                rb_sb = cpool.tile([P, E], f32, tag="rb")
                nc.sync.dma_start(rb_sb[:], rb_d[:])
                ones_sb = cpool.tile([P, 1], f32, tag="ones")
                nc.vector.memset(ones_sb[:], 1.0)

                # ---------------- router (data-parallel over tokens) --------
                s_acc = cpool.tile([1, 2 * E], f32, tag="s_acc")
                nc.vector.memset(s_acc[:], 0.0)
                for t in range(ntr):
                    xr_sb = rpool.tile([P, KH, P], f32, tag="xr")
                    nc.sync.dma_start(xr_sb[:], xr_d[:, :, t * P:(t + 1) * P])
                    lgp = ps_r.tile([P, E], f32, tag="lg")
                    for k in range(KH):
                        nc.tensor.matmul(
                            lgp[:], xr_sb[:, k, :], rwt_sb[:, k, :],
                            start=(k == 0), stop=(k == KH - 1),
                        )
                    lgs = rpool.tile([P, E], f32, tag="lgs")
                    nc.vector.tensor_tensor(out=lgs[:], in0=lgp[:], in1=rb_sb[:], op=OP.add)
                    nc.sync.dma_start(lg_d[t * P:(t + 1) * P, :], lgs[:])
                    # softmax over E
                    m1 = rpool.tile([P, 1], f32, tag="m1")
                    nc.vector.tensor_reduce(m1[:], lgs[:], axis=X, op=OP.max)
                    sub = rpool.tile([P, E], f32, tag="sub")
                    nc.vector.tensor_scalar(out=sub[:], in0=lgs[:], scalar1=m1[:, 0:1],
                                            scalar2=None, op0=OP.subtract)
                    ex = rpool.tile([P, E], f32, tag="ex")
                    nc.scalar.activation(ex[:], sub[:], AF.Exp)
                    ssum = rpool.tile([P, 1], f32, tag="ssum")
                    nc.vector.tensor_reduce(ssum[:], ex[:], axis=X, op=OP.add)
                    rinv = rpool.tile([P, 1], f32, tag="rinv")
                    nc.vector.reciprocal(rinv[:], ssum[:])
                    smx = rpool.tile([P, E], f32, tag="smx")
                    nc.vector.tensor_scalar(out=smx[:], in0=ex[:], scalar1=rinv[:, 0:1],
                                            scalar2=None, op0=OP.mult)
                    nc.sync.dma_start(sm_d[t * P:(t + 1) * P, :], smx[:])
                    # top-2 dispatch mask: g1 at argmax1, g2 at argmax2
                    is1 = rpool.tile([P, E], f32, tag="is1")
                    nc.vector.tensor_scalar(out=is1[:], in0=lgs[:], scalar1=m1[:, 0:1],
                                            scalar2=None, op0=OP.is_equal)
                    mskd = rpool.tile([P, E], f32, tag="mskd")
                    nc.vector.scalar_tensor_tensor(out=mskd[:], in0=is1[:], scalar=-1e30,
                                                   in1=lgs[:], op0=OP.mult, op1=OP.add)
                    m2 = rpool.tile([P, 1], f32, tag="m2")
                    nc.vector.tensor_reduce(m2[:], mskd[:], axis=X, op=OP.max)
                    is2 = rpool.tile([P, E], f32, tag="is2")
                    nc.vector.tensor_scalar(out=is2[:], in0=mskd[:], scalar1=m2[:, 0:1],
                                            scalar2=None, op0=OP.is_equal)
                    d12 = rpool.tile([P, 1], f32, tag="d12")
                    nc.vector.tensor_tensor(out=d12[:], in0=m1[:], in1=m2[:], op=OP.subtract)
                    g1 = rpool.tile([P, 1], f32, tag="g1")
                    nc.scalar.activation(g1[:], d12[:], AF.Sigmoid)
                    g2 = rpool.tile([P, 1], f32, tag="g2")
                    nc.vector.tensor_scalar(out=g2[:], in0=g1[:], scalar1=-1.0, scalar2=1.0,
                                            op0=OP.mult, op1=OP.add)
                    mk1 = rpool.tile([P, E], f32, tag="mk1")
                    nc.vector.tensor_scalar(out=mk1[:], in0=is1[:], scalar1=g1[:, 0:1],
                                            scalar2=None, op0=OP.mult)
                    mks = rpool.tile([P, E], f32, tag="mks")
                    nc.vector.scalar_tensor_tensor(out=mks[:], in0=is2[:], scalar=g2[:, 0:1],
                                                   in1=mk1[:], op0=OP.mult, op1=OP.add)
                    nc.sync.dma_start(mk_d[t * P:(t + 1) * P, :], mks[:])
                    # column sums over this tile's 128 tokens (ones-matmul)
                    st = ps_r.tile([1, 2 * E], f32, tag="st")
                    nc.tensor.matmul(st[0:1, 0:E], ones_sb[:, 0:1], smx[:],
                                     start=True, stop=True)
                    nc.tensor.matmul(st[0:1, E:2 * E], ones_sb[:, 0:1], mks[:],
                                     start=True, stop=True)
                    nc.vector.tensor_tensor(out=s_acc[:], in0=s_acc[:], in1=st[:], op=OP.add)
                nc.sync.dma_start(s_d[:], s_acc[:])

                # ---------------- expert FFN (this core's expert) -----------
                # gelu-tanh constants: 0.5*x*(1+tanh(c0*x + c1*x^3))
                C0 = 0.7978845608028654
                C1 = C0 * 0.044715
                for c in range(nch):
                    xc = xpool.tile([P, KH, CHUNK], mdt, tag="xc")
                    nc.sync.dma_start(xc[:], xT_ffn[:, :, c * CHUNK:(c + 1) * CHUNK])
                    hT = hpool.tile([P, KF, CHUNK], mdt, tag="hT")
                    for m in range(KF):
                        hp = ps_h.tile([P, CHUNK], f32, tag="hp")
                        for k in range(KH):
                            nc.tensor.matmul(
                                hp[:], w1_sb[:, k, m * P:(m + 1) * P], xc[:, k, :],
                                start=(k == 0), stop=(k == KH - 1),
                            )
                        # xb = hp + b1 ; gelu via tanh approximation
                        xb = ypool.tile([P, CHUNK], f32, tag="xb")
                        nc.scalar.activation(xb[:], hp[:], AF.Identity,
                                             bias=b1_sb[:, m:m + 1])
                        sq = ypool.tile([P, CHUNK], f32, tag="sq")
                        nc.scalar.activation(sq[:], xb[:], AF.Square)
                        poly = ypool.tile([P, CHUNK], f32, tag="poly")
                        nc.vector.tensor_scalar(out=poly[:], in0=sq[:], scalar1=C1,
                                                scalar2=C0, op0=OP.mult, op1=OP.add)
                        nc.vector.tensor_tensor(out=poly[:], in0=poly[:], in1=xb[:],
                                                op=OP.mult)
                        th = ypool.tile([P, CHUNK], f32, tag="th")
                        nc.scalar.activation(th[:], poly[:], AF.Tanh)
                        nc.vector.tensor_scalar(out=th[:], in0=th[:], scalar1=1.0,
                                                scalar2=0.5, op0=OP.add, op1=OP.mult)
                        nc.vector.tensor_tensor(out=hT[:, m, :], in0=th[:], in1=xb[:],
                                                op=OP.mult)
                    for tt in range(CHUNK // P):
                        yp = ps_y.tile([P, H], f32, tag="yp")
                        for k in range(KF):
                            nc.tensor.matmul(yp[:, 0:512], hT[:, k, tt * P:(tt + 1) * P],
                                             w2_sb[:, k, 0:512],
                                             start=(k == 0), stop=(k == KF - 1))
                            nc.tensor.matmul(yp[:, 512:H], hT[:, k, tt * P:(tt + 1) * P],
                                             w2_sb[:, k, 512:H],
                                             start=(k == 0), stop=(k == KF - 1))
                        ys = ypool.tile([P, H], f32, tag="ys")
                        nc.vector.tensor_tensor(out=ys[:], in0=yp[:], in1=b2_sb[:], op=OP.add)
                        j = c * (CHUNK // P) + tt
                        nc.vector.tensor_scalar(out=ys[:], in0=ys[:], scalar1=g_sb[:, j:j + 1],
                                                scalar2=None, op0=OP.mult)
                        nc.sync.dma_start(y_d[j * P:(j + 1) * P, :], ys[:])

    nc.compile()
    return nc


def _xT_blocks(xmat, kblocks):
    """[N, kblocks*P] row-major -> [P, kblocks, N] (transposed block layout)."""
    n = xmat.shape[0]
    return np.ascontiguousarray(
        xmat.T.reshape(kblocks, P, n).transpose(1, 0, 2)
    )


def host_route(x, router_w, router_b):
    """numpy replica of the routing decision (indices + gates for dispatch)."""
    logits = (x @ router_w + router_b).astype(np.float32)
    idx1 = np.argmax(logits, axis=1)
    rows = np.arange(x.shape[0])
    masked = logits.copy()
    masked[rows, idx1] = -np.inf
    idx2 = np.argmax(masked, axis=1)
    v1 = logits[rows, idx1]
    v2 = logits[rows, idx2]
    g1 = (1.0 / (1.0 + np.exp((v2 - v1).astype(np.float64)))).astype(np.float32)
    g2 = np.float32(1.0) - g1
    return logits, idx1, idx2, g1, g2


def kernel(hidden_states, router_w, router_b, w1, b1, w2, b2):
    hidden_states = np.asarray(hidden_states, dtype=np.float32)
    router_w = np.asarray(router_w, dtype=np.float32)
    router_b = np.asarray(router_b, dtype=np.float32)
    w1 = np.asarray(w1, dtype=np.float32)
    b1 = np.asarray(b1, dtype=np.float32)
    w2 = np.asarray(w2, dtype=np.float32)
    b2 = np.asarray(b2, dtype=np.float32)

    B, S, Hd = hidden_states.shape
    assert Hd == H
    T = B * S
    assert T % N_CORES == 0
    TS = T // N_CORES
    x = np.ascontiguousarray(hidden_states.reshape(T, H))

    # ---- host routing decision (for the expert-parallel gather only) ----
    logits_h, idx1, idx2, g1, g2 = host_route(x, router_w, router_b)

    tok_lists, gate_lists = [], []
    for e in range(E):
        s1 = idx1 == e
        s2 = idx2 == e
        tok = np.nonzero(s1 | s2)[0]
        gate = np.where(s1[tok], g1[tok], g2[tok]).astype(np.float32)
        tok_lists.append(tok)
        gate_lists.append(gate)
    maxcnt = max(len(t) for t in tok_lists)
    C = int(-(-maxcnt // CHUNK) * CHUNK)

    reps = int(os.environ.get("MOE_REPS", "1"))
    mm_dtype = os.environ.get("MOE_MM_DTYPE", "float32r")
    key = (C, TS, reps, mm_dtype)
    if key not in _PROGRAM_CACHE:
        _PROGRAM_CACHE[key] = build_program(C, TS, reps=reps, mm_dtype_name=mm_dtype)
    nc = _PROGRAM_CACHE[key]

    import ml_dtypes
    np_mdt = np.float32 if mm_dtype == "float32r" else ml_dtypes.bfloat16

    rwt_arr = np.ascontiguousarray(
        router_w.reshape(KH, P, E).transpose(1, 0, 2))
    rb_arr = np.ascontiguousarray(np.broadcast_to(router_b, (P, E)))

    in_maps = []
    for e in range(E):
        tok = tok_lists[e]
        n = len(tok)
        xg = np.zeros((C, H), dtype=np.float32)
        xg[:n] = x[tok]
        gates = np.zeros((C,), dtype=np.float32)
        gates[:n] = gate_lists[e]
        xs = x[e * TS:(e + 1) * TS]
        in_maps.append({
            "xT_ffn": _xT_blocks(xg, KH).astype(np_mdt),
            "w1": np.ascontiguousarray(
                w1[e].reshape(KH, P, F).transpose(1, 0, 2)).astype(np_mdt),
            "w2": np.ascontiguousarray(
                w2[e].reshape(KF, P, H).transpose(1, 0, 2)).astype(np_mdt),
            "b1": np.ascontiguousarray(b1[e].reshape(KF, P).T),
            "b2": np.ascontiguousarray(np.broadcast_to(b2[e], (P, H))),
            "gates": np.ascontiguousarray(gates.reshape(C // P, P).T),
            "x_r": _xT_blocks(xs, KH),
            "rwght": rwt_arr,
            "rbias": rb_arr,
        })

    from concourse.bass_utils import run_bass_kernel_spmd
    res = run_bass_kernel_spmd(nc, in_maps, core_ids=list(range(N_CORES)))
    results = res.results

    # ---- gather / unshard ----
    combined = np.zeros((T, H), dtype=np.float32)
    for e in range(E):
        tok = tok_lists[e]
        combined[tok] += results[e]["y"][:len(tok)]

    logits = np.concatenate([results[i]["logits"] for i in range(N_CORES)], axis=0)
    softmax = np.concatenate([results[i]["softmax"] for i in range(N_CORES)], axis=0)
    mask = np.concatenate([results[i]["mask"] for i in range(N_CORES)], axis=0)
    sums = np.sum([results[i]["sums"] for i in range(N_CORES)], axis=0)[0]
    aux = np.float32(E) * np.float32(
        np.dot(sums[0:E] / np.float32(T), sums[E:2 * E] / np.float32(T)))

    return (
        combined.reshape(B, S, H),
        softmax.reshape(B, S, E),
        mask.reshape(B, S, E),
        np.float32(aux),
        logits.reshape(B, S, E),
    )


# revision 23
# speedup vs baseline: 1372.6413x; 1372.6413x over previous
"""Trainium2 Bass kernel for an 8-expert top-2 MoE layer (768 hidden, 3072 FFN).

Strategy (expert-parallel over 8 NeuronCores):
  - Each core owns one expert's FFN weights (w1[e], b1[e], w2[e], b2[e]).
  - The host computes routing indices (which tokens go to which expert) and
    gathers/pads each expert's tokens to a fixed capacity C; the device
    computes (gelu(x@w1+b1)@w2+b2)*gate for those tokens.
  - The router itself (logits, softmax, top-2 dispatch mask) is ALSO computed
    on device, data-parallel: core i handles tokens [i*T/8, (i+1)*T/8).
    Router tiles are interleaved between FFN chunks so their vector/scalar
    work hides under the FFN matmuls.
  - The host scatters the per-expert outputs back (each token receives
    exactly TOP_K=2 contributions) and reduces the aux loss from the
    returned routing outputs.

All matmuls use float32r (fp32 data at ~1 cycle/row on the PE when the
moving free dim >= 256).
"""

import os
import numpy as np

P = 128
E = 8
H = 768
F = 3072
KH = H // P      # 6
KF = F // P      # 24
CHUNK = 256      # tokens per mm1 moving block (>=256 keeps float32r full-rate)
N_CORES = 8

_PROGRAM_CACHE = {}


def build_program(C, TS, reps=1, mm_dtype_name="float32r", gelu_mode="lut",
                  b2_zero=True, sections=("router", "ffn")):
    """Build + compile the per-core SPMD Bass program.

    C:  padded per-expert token capacity (multiple of CHUNK)
    TS: tokens per core for the router section (T / 8)
    reps: repeat the whole body (for wall-clock delta timing); outputs are
          rewritten identically each rep.
    """
    import concourse.mybir as mybir
    import concourse.tile as tile
    from concourse import bacc

    f32 = mybir.dt.float32
    mdt = getattr(mybir.dt, mm_dtype_name)
    AF = mybir.ActivationFunctionType
    OP = mybir.AluOpType
    X = mybir.AxisListType.X

    assert C % CHUNK == 0 and TS % P == 0
    nch = C // CHUNK
    ntr = TS // P

    nc = bacc.Bacc(None, target_bir_lowering=False)

    xT_ffn = nc.dram_tensor("xT_ffn", [P, KH, C], mdt, kind="ExternalInput")
    w1_d = nc.dram_tensor("w1", [P, KH, F], mdt, kind="ExternalInput")
    w2_d = nc.dram_tensor("w2", [P, KF, H], mdt, kind="ExternalInput")
    b1_d = nc.dram_tensor("b1", [P, KF], f32, kind="ExternalInput")
    b2_d = None
    if not b2_zero:
        b2_d = nc.dram_tensor("b2", [P, H], f32, kind="ExternalInput")
    g_d = nc.dram_tensor("gates", [P, C // P], f32, kind="ExternalInput")
    xr_d = nc.dram_tensor("x_r", [P, KH, TS], f32, kind="ExternalInput")
    rwt_d = nc.dram_tensor("rwght", [P, KH, E], f32, kind="ExternalInput")
    rb_d = nc.dram_tensor("rbias", [P, E], f32, kind="ExternalInput")

    y_d = nc.dram_tensor("y", [C, H], f32, kind="ExternalOutput")
    lg_d = nc.dram_tensor("logits", [TS, E], f32, kind="ExternalOutput")
    sm_d = nc.dram_tensor("softmax", [TS, E], f32, kind="ExternalOutput")
    mk_d = nc.dram_tensor("mask", [TS, E], f32, kind="ExternalOutput")

    # gelu-tanh constants: 0.5*x*(1+tanh(c0*x + c1*x^3))
    C0 = 0.7978845608028654
    C1 = C0 * 0.044715

    with tile.TileContext(nc) as tc:
        with (
            tc.tile_pool(name="wpool", bufs=1) as wpool,
            tc.tile_pool(name="cpool", bufs=1) as cpool,
            tc.tile_pool(name="xpool", bufs=2) as xpool,
            tc.tile_pool(name="hpool", bufs=1) as hpool,
            tc.tile_pool(name="ypool", bufs=2) as ypool,
            tc.tile_pool(name="rpool", bufs=2) as rpool,
            tc.tile_pool(name="ps_h", bufs=2, space="PSUM") as ps_h,
            tc.tile_pool(name="ps_y", bufs=2, space="PSUM") as ps_y,
            tc.tile_pool(name="ps_r", bufs=2, space="PSUM") as ps_r,
        ):
            def load_xr(t):
                xr_sb = rpool.tile([P, KH, P], f32, tag="xr")
                nc.sync.dma_start(xr_sb[:], xr_d[:, :, t * P:(t + 1) * P])
                return xr_sb

            def load_xc(c):
                xc = xpool.tile([P, KH, CHUNK], mdt, tag="xc")
                nc.sync.dma_start(xc[:], xT_ffn[:, :, c * CHUNK:(c + 1) * CHUNK])
                return xc

            def router_tile(t, rwt_sb, rb_sb, xr_sb=None):
                if xr_sb is None:
                    xr_sb = load_xr(t)
                lgp = ps_r.tile([P, E], f32, tag="lg")
                for k in range(KH):
                    nc.tensor.matmul(
                        lgp[:], xr_sb[:, k, :], rwt_sb[:, k, :],
                        start=(k == 0), stop=(k == KH - 1),
                    )
                lgs = rpool.tile([P, E], f32, tag="lgs")
                nc.vector.tensor_tensor(out=lgs[:], in0=lgp[:], in1=rb_sb[:],
                                        op=OP.add)
                nc.sync.dma_start(lg_d[t * P:(t + 1) * P, :], lgs[:])
                # softmax over E
                m1 = rpool.tile([P, 1], f32, tag="m1")
                nc.vector.tensor_reduce(m1[:], lgs[:], axis=X, op=OP.max)
                sub = rpool.tile([P, E], f32, tag="sub")
                nc.vector.tensor_scalar(out=sub[:], in0=lgs[:], scalar1=m1[:, 0:1],
                                        scalar2=None, op0=OP.subtract)
                ex = rpool.tile([P, E], f32, tag="ex")
                nc.scalar.activation(ex[:], sub[:], AF.Exp)
                ssum = rpool.tile([P, 1], f32, tag="ssum")
                nc.vector.tensor_reduce(ssum[:], ex[:], axis=X, op=OP.add)
                rinv = rpool.tile([P, 1], f32, tag="rinv")
                nc.vector.reciprocal(rinv[:], ssum[:])
                smx = rpool.tile([P, E], f32, tag="smx")
                nc.vector.tensor_scalar(out=smx[:], in0=ex[:], scalar1=rinv[:, 0:1],
                                        scalar2=None, op0=OP.mult)
                nc.sync.dma_start(sm_d[t * P:(t + 1) * P, :], smx[:])
                # top-2 dispatch mask: g1 at argmax1, g2 at argmax2
                is1 = rpool.tile([P, E], f32, tag="is1")
                nc.vector.tensor_scalar(out=is1[:], in0=lgs[:], scalar1=m1[:, 0:1],
                                        scalar2=None, op0=OP.is_equal)
                mskd = rpool.tile([P, E], f32, tag="mskd")
                nc.vector.scalar_tensor_tensor(out=mskd[:], in0=is1[:], scalar=-1e30,
                                               in1=lgs[:], op0=OP.mult, op1=OP.add)
                m2 = rpool.tile([P, 1], f32, tag="m2")
                nc.vector.tensor_reduce(m2[:], mskd[:], axis=X, op=OP.max)
                is2 = rpool.tile([P, E], f32, tag="is2")
                nc.vector.tensor_scalar(out=is2[:], in0=mskd[:], scalar1=m2[:, 0:1],
                                        scalar2=None, op0=OP.is_equal)
                # g1 = sigmoid(m1-m2) as 1/(1+exp(m2-m1)): ACT only needs Exp
                d12 = rpool.tile([P, 1], f32, tag="d12")
                nc.vector.tensor_tensor(out=d12[:], in0=m2[:], in1=m1[:],
                                        op=OP.subtract)
                e12 = rpool.tile([P, 1], f32, tag="e12")
                nc.scalar.activation(e12[:], d12[:], AF.Exp)
                nc.vector.tensor_scalar(out=e12[:], in0=e12[:], scalar1=1.0,
                                        scalar2=None, op0=OP.add)
                g1 = rpool.tile([P, 1], f32, tag="g1")
                nc.vector.reciprocal(g1[:], e12[:])
                g2 = rpool.tile([P, 1], f32, tag="g2")
                nc.vector.tensor_scalar(out=g2[:], in0=g1[:], scalar1=-1.0,
                                        scalar2=1.0, op0=OP.mult, op1=OP.add)
                mk1 = rpool.tile([P, E], f32, tag="mk1")
                nc.vector.tensor_scalar(out=mk1[:], in0=is1[:], scalar1=g1[:, 0:1],
                                        scalar2=None, op0=OP.mult)
                mks = rpool.tile([P, E], f32, tag="mks")
                nc.vector.scalar_tensor_tensor(out=mks[:], in0=is2[:],
                                               scalar=g2[:, 0:1], in1=mk1[:],
                                               op0=OP.mult, op1=OP.add)
                nc.sync.dma_start(mk_d[t * P:(t + 1) * P, :], mks[:])

            def ffn_chunk(c, w1_sb, w2_sb, b1_sb, b2_sb, g_sb, xc=None):
                if xc is None:
                    xc = load_xc(c)
                # two half-tiles so next chunk's gelu can reuse half A while
                # this chunk's mm2 is still reading half B
                hTa = hpool.tile([P, KF // 2, CHUNK], mdt, tag="hTa")
                hTb = hpool.tile([P, KF // 2, CHUNK], mdt, tag="hTb")

                def hT(k):
                    return hTa[:, k, :] if k < KF // 2 else hTb[:, k - KF // 2, :]

                for m in range(KF):
                    hp = ps_h.tile([P, CHUNK], f32, tag="hp")
                    for k in range(KH):
                        nc.tensor.matmul(
                            hp[:], w1_sb[:, k, m * P:(m + 1) * P], xc[:, k, :],
                            start=(k == 0), stop=(k == KH - 1),
                        )
                    if gelu_mode == "lut":
                        nc.scalar.activation(hT(m), hp[:],
                                             AF.Gelu_apprx_tanh,
                                             bias=b1_sb[:, m:m + 1])
                    else:
                        # explicit tanh gelu (CoreSim-friendly)
                        xb = ypool.tile([P, CHUNK], f32, tag="xb")
                        nc.scalar.activation(xb[:], hp[:], AF.Identity,
                                             bias=b1_sb[:, m:m + 1])
                        sq = ypool.tile([P, CHUNK], f32, tag="sq")
                        nc.scalar.activation(sq[:], xb[:], AF.Square)
                        poly = ypool.tile([P, CHUNK], f32, tag="poly")
                        nc.vector.tensor_scalar(out=poly[:], in0=sq[:], scalar1=C1,
                                                scalar2=C0, op0=OP.mult, op1=OP.add)
                        nc.vector.tensor_tensor(out=poly[:], in0=poly[:], in1=xb[:],
                                                op=OP.mult)
                        th = ypool.tile([P, CHUNK], f32, tag="th")
                        nc.scalar.activation(th[:], poly[:], AF.Tanh)
                        nc.vector.tensor_scalar(out=th[:], in0=th[:], scalar1=1.0,
                                                scalar2=0.5, op0=OP.add, op1=OP.mult)
                        nc.vector.tensor_tensor(out=hT(m), in0=th[:],
                                                in1=xb[:], op=OP.mult)
                for tt in range(CHUNK // P):
                    yp = ps_y.tile([P, H], f32, tag="yp")
                    for k in range(KF):
                        hTk = hT(k)[:, tt * P:(tt + 1) * P]
                        nc.tensor.matmul(yp[:, 0:512], hTk,
                                         w2_sb[:, k, 0:512],
                                         start=(k == 0), stop=(k == KF - 1))
                        nc.tensor.matmul(yp[:, 512:H], hTk,
                                         w2_sb[:, k, 512:H],
                                         start=(k == 0), stop=(k == KF - 1))
                    ys = ypool.tile([P, H], f32, tag="ys")
                    j = c * (CHUNK // P) + tt
                    if b2_zero:
                        nc.vector.tensor_scalar(out=ys[:], in0=yp[:],
                                                scalar1=g_sb[:, j:j + 1],
                                                scalar2=None, op0=OP.mult)
                    else:
                        nc.vector.tensor_tensor(out=ys[:], in0=yp[:],
                                                in1=b2_sb[:], op=OP.add)
                        nc.vector.tensor_scalar(out=ys[:], in0=ys[:],
                                                scalar1=g_sb[:, j:j + 1],
                                                scalar2=None, op0=OP.mult)
                    nc.sync.dma_start(y_d[j * P:(j + 1) * P, :], ys[:])

            for _rep in range(reps):
                nts = ntr if "router" in sections else 0
                ncs = nch if "ffn" in sections else 0

                # --- DMA priority order: tiny consts + first working set
                # first, then the big weight streams (in the order the PE
                # consumes them), so the PE starts within ~10us instead of
                # waiting for the full 19MB weight load.
                b1_sb = cpool.tile([P, KF], f32, tag="b1")
                nc.sync.dma_start(b1_sb[:], b1_d[:])
                b2_sb = None
                if not b2_zero:
                    b2_sb = cpool.tile([P, H], f32, tag="b2")
                    nc.sync.dma_start(b2_sb[:], b2_d[:])
                g_sb = cpool.tile([P, C // P], f32, tag="g")
                nc.sync.dma_start(g_sb[:], g_d[:])
                rwt_sb = cpool.tile([P, KH, E], f32, tag="rwt")
                nc.sync.dma_start(rwt_sb[:], rwt_d[:])
                rb_sb = cpool.tile([P, E], f32, tag="rb")
                nc.sync.dma_start(rb_sb[:], rb_d[:])
                xr0 = load_xr(0) if nts else None
                xc0 = load_xc(0) if ncs else None

                # w1 streamed in column blocks (mm1 consumes columns in
                # order); w2 streamed in k blocks (mm2 consumption order)
                w1_sb = wpool.tile([P, KH, F], mdt, tag="w1")
                nc.sync.dma_start(w1_sb[:, :, 0:128], w1_d[:, :, 0:128])
                nc.sync.dma_start(w1_sb[:, :, 128:384], w1_d[:, :, 128:384])
                for ms in range(384, F, 384):
                    nc.sync.dma_start(w1_sb[:, :, ms:ms + 384],
                                      w1_d[:, :, ms:ms + 384])
                w2_sb = wpool.tile([P, KF, H], mdt, tag="w2")
                for k in range(0, KF, 2):
                    nc.sync.dma_start(w2_sb[:, k:k + 2, :], w2_d[:, k:k + 2, :])

                # interleave: one router token-tile between FFN chunks so the
                # router's vector/scalar chain hides under the FFN matmuls
                for c in range(max(ncs, nts)):
                    if c < nts:
                        router_tile(c, rwt_sb, rb_sb, xr_sb=xr0 if c == 0 else None)
                    if c < ncs:
                        ffn_chunk(c, w1_sb, w2_sb, b1_sb, b2_sb, g_sb,
                                  xc=xc0 if c == 0 else None)

    nc.compile()
    return nc


def _xT_blocks(xmat, kblocks):
    """[N, kblocks*P] row-major -> [P, kblocks, N] (transposed block layout)."""
    n = xmat.shape[0]
    return np.ascontiguousarray(
        xmat.T.reshape(kblocks, P, n).transpose(1, 0, 2)
    )


def host_route(x, router_w, router_b):
    """numpy replica of the routing decision (indices + gates for dispatch)."""
    logits = (x @ router_w + router_b).astype(np.float32)
    idx1 = np.argmax(logits, axis=1)
    rows = np.arange(x.shape[0])
    masked = logits.copy()
    masked[rows, idx1] = -np.inf
    idx2 = np.argmax(masked, axis=1)
    v1 = logits[rows, idx1]
    v2 = logits[rows, idx2]
    g1 = (1.0 / (1.0 + np.exp((v2 - v1).astype(np.float64)))).astype(np.float32)
    g2 = np.float32(1.0) - g1
    return logits, idx1, idx2, g1, g2


def kernel(hidden_states, router_w, router_b, w1, b1, w2, b2):
    hidden_states = np.asarray(hidden_states, dtype=np.float32)
    router_w = np.asarray(router_w, dtype=np.float32)
    router_b = np.asarray(router_b, dtype=np.float32)
    w1 = np.asarray(w1, dtype=np.float32)
    b1 = np.asarray(b1, dtype=np.float32)
    w2 = np.asarray(w2, dtype=np.float32)
    b2 = np.asarray(b2, dtype=np.float32)

    try:
        import jax
        jax.config.update("jax_compilation_cache_dir", "/tmp/jax_cache")
        jax.config.update("jax_persistent_cache_min_compile_time_secs", 1.0)
    except Exception:
        pass

    B, S, Hd = hidden_states.shape
    assert Hd == H
    T = B * S
    assert T % N_CORES == 0
    TS = T // N_CORES
    x = np.ascontiguousarray(hidden_states.reshape(T, H))

    # ---- host routing decision (for the expert-parallel gather only) ----
    logits_h, idx1, idx2, g1, g2 = host_route(x, router_w, router_b)

    tok_lists, gate_lists = [], []
    for e in range(E):
        s1 = idx1 == e
        s2 = idx2 == e
        tok = np.nonzero(s1 | s2)[0]
        gate = np.where(s1[tok], g1[tok], g2[tok]).astype(np.float32)
        tok_lists.append(tok)
        gate_lists.append(gate)
    maxcnt = max(len(t) for t in tok_lists)
    C = int(-(-maxcnt // CHUNK) * CHUNK)

    reps = int(os.environ.get("MOE_REPS", "1"))
    mm_dtype = os.environ.get("MOE_MM_DTYPE", "float32r")
    gelu_mode = os.environ.get("MOE_GELU", "lut")
    b2_zero = bool(np.all(b2 == 0))
    key = (C, TS, reps, mm_dtype, gelu_mode, b2_zero)
    if key not in _PROGRAM_CACHE:
        _PROGRAM_CACHE[key] = build_program(
            C, TS, reps=reps, mm_dtype_name=mm_dtype, gelu_mode=gelu_mode,
            b2_zero=b2_zero)
    nc = _PROGRAM_CACHE[key]

    import ml_dtypes
    np_mdt = np.float32 if mm_dtype == "float32r" else ml_dtypes.bfloat16

    rwt_arr = np.ascontiguousarray(
        router_w.reshape(KH, P, E).transpose(1, 0, 2))
    rb_arr = np.ascontiguousarray(np.broadcast_to(router_b, (P, E)))

    in_maps = []
    for e in range(E):
        tok = tok_lists[e]
        n = len(tok)
        xg = np.zeros((C, H), dtype=np.float32)
        xg[:n] = x[tok]
        gates = np.zeros((C,), dtype=np.float32)
        gates[:n] = gate_lists[e]
        xs = x[e * TS:(e + 1) * TS]
        im = {
            "xT_ffn": _xT_blocks(xg, KH).astype(np_mdt),
            "w1": np.ascontiguousarray(
                w1[e].reshape(KH, P, F).transpose(1, 0, 2)).astype(np_mdt),
            "w2": np.ascontiguousarray(
                w2[e].reshape(KF, P, H).transpose(1, 0, 2)).astype(np_mdt),
            "b1": np.ascontiguousarray(b1[e].reshape(KF, P).T),
            "gates": np.ascontiguousarray(gates.reshape(C // P, P).T),
            "x_r": _xT_blocks(xs, KH),
            "rwght": rwt_arr,
            "rbias": rb_arr,
        }
        if not b2_zero:
            im["b2"] = np.ascontiguousarray(np.broadcast_to(b2[e], (P, H)))
        in_maps.append(im)

    from concourse.bass_utils import run_bass_kernel_spmd
    res = run_bass_kernel_spmd(nc, in_maps, core_ids=list(range(N_CORES)))
    results = res.results

    # ---- gather / unshard ----
    combined = np.zeros((T, H), dtype=np.float32)
    for e in range(E):
        tok = tok_lists[e]
        combined[tok] += results[e]["y"][:len(tok)]

    logits = np.concatenate([results[i]["logits"] for i in range(N_CORES)], axis=0)
    softmax = np.concatenate([results[i]["softmax"] for i in range(N_CORES)], axis=0)
    mask = np.concatenate([results[i]["mask"] for i in range(N_CORES)], axis=0)
    mean_prob = softmax.mean(axis=0, dtype=np.float64)
    tokens_per_expert = mask.sum(axis=0, dtype=np.float64) / T
    aux = np.float32(E * np.dot(mean_prob, tokens_per_expert))

    return (
        combined.reshape(B, S, H),
        softmax.reshape(B, S, E),
        mask.reshape(B, S, E),
        np.float32(aux),
        logits.reshape(B, S, E),
    )


# revision 26
# speedup vs baseline: 1466.5509x; 1.0684x over previous
"""Trainium2 Bass kernel for an 8-expert top-2 MoE layer (768 hidden, 3072 FFN).

Strategy (expert-parallel over 8 NeuronCores):
  - Each core owns one expert's FFN weights (w1[e], b1[e], w2[e], b2[e]).
  - The host computes routing indices (which tokens go to which expert) and
    gathers/pads each expert's tokens to a fixed capacity C; the device
    computes (gelu(x@w1+b1)@w2+b2)*gate for those tokens.
  - The router itself (logits, softmax, top-2 dispatch mask) is ALSO computed
    on device, data-parallel: core i handles tokens [i*T/8, (i+1)*T/8).
    Router tiles are interleaved between FFN chunks so their vector/scalar
    work hides under the FFN matmuls.
  - The host scatters the per-expert outputs back (each token receives
    exactly TOP_K=2 contributions) and reduces the aux loss from the
    returned routing outputs.

All matmuls use float32r (fp32 data at ~1 cycle/row on the PE when the
moving free dim >= 256).
"""

import os
import numpy as np

P = 128
E = 8
H = 768
F = 3072
KH = H // P      # 6
KF = F // P      # 24
CHUNK = 256      # tokens per mm1 moving block (>=256 keeps float32r full-rate)
N_CORES = 8

_PROGRAM_CACHE = {}


def build_program(C, TS, reps=1, mm_dtype_name="float32r", gelu_mode="lut",
                  b2_zero=True, sections=("router", "ffn")):
    """Build + compile the per-core SPMD Bass program.

    C:  padded per-expert token capacity (multiple of CHUNK)
    TS: tokens per core for the router section (T / 8)
    reps: repeat the whole body (for wall-clock delta timing); outputs are
          rewritten identically each rep.
    """
    import concourse.mybir as mybir
    import concourse.tile as tile
    from concourse import bacc

    f32 = mybir.dt.float32
    mdt = getattr(mybir.dt, mm_dtype_name)
    AF = mybir.ActivationFunctionType
    OP = mybir.AluOpType
    X = mybir.AxisListType.X

    assert C % CHUNK == 0 and TS % P == 0
    nch = C // CHUNK
    ntr = TS // P

    nc = bacc.Bacc(None, target_bir_lowering=False)

    xT_ffn = nc.dram_tensor("xT_ffn", [P, KH, C], mdt, kind="ExternalInput")
    w1_d = nc.dram_tensor("w1", [P, KH, F], mdt, kind="ExternalInput")
    w2_d = nc.dram_tensor("w2", [P, KF, H], mdt, kind="ExternalInput")
    b1_d = nc.dram_tensor("b1", [P, KF], f32, kind="ExternalInput")
    b2_d = None
    if not b2_zero:
        b2_d = nc.dram_tensor("b2", [P, H], f32, kind="ExternalInput")
    g_d = nc.dram_tensor("gates", [P, C // P], f32, kind="ExternalInput")
    xr_d = nc.dram_tensor("x_r", [P, KH, TS], f32, kind="ExternalInput")
    rwt_d = nc.dram_tensor("rwght", [P, KH, E], f32, kind="ExternalInput")
    rb_d = nc.dram_tensor("rbias", [P, E], f32, kind="ExternalInput")

    y_d = nc.dram_tensor("y", [C, H], f32, kind="ExternalOutput")
    lg_d = nc.dram_tensor("logits", [TS, E], f32, kind="ExternalOutput")
    sm_d = nc.dram_tensor("softmax", [TS, E], f32, kind="ExternalOutput")
    mk_d = nc.dram_tensor("mask", [TS, E], f32, kind="ExternalOutput")

    # gelu-tanh constants: 0.5*x*(1+tanh(c0*x + c1*x^3))
    C0 = 0.7978845608028654
    C1 = C0 * 0.044715

    with tile.TileContext(nc) as tc:
        with (
            tc.tile_pool(name="wpool", bufs=1) as wpool,
            tc.tile_pool(name="cpool", bufs=1) as cpool,
            tc.tile_pool(name="xpool", bufs=2) as xpool,
            tc.tile_pool(name="hpool", bufs=1) as hpool,
            tc.tile_pool(name="ypool", bufs=2) as ypool,
            tc.tile_pool(name="rpool", bufs=2) as rpool,
            tc.tile_pool(name="ps_h", bufs=3, space="PSUM") as ps_h,
            tc.tile_pool(name="ps_y", bufs=2, space="PSUM") as ps_y,
            tc.tile_pool(name="ps_r", bufs=1, space="PSUM") as ps_r,
        ):
            def load_xr(t):
                xr_sb = rpool.tile([P, KH, P], f32, tag="xr")
                nc.sync.dma_start(xr_sb[:], xr_d[:, :, t * P:(t + 1) * P])
                return xr_sb

            def load_xc(c):
                xc = xpool.tile([P, KH, CHUNK], mdt, tag="xc")
                nc.sync.dma_start(xc[:], xT_ffn[:, :, c * CHUNK:(c + 1) * CHUNK])
                return xc

            def router_tile(t, rwt_sb, rb_sb, xr_sb=None):
                if xr_sb is None:
                    xr_sb = load_xr(t)
                lgp = ps_r.tile([P, E], f32, tag="lg")
                for k in range(KH):
                    nc.tensor.matmul(
                        lgp[:], xr_sb[:, k, :], rwt_sb[:, k, :],
                        start=(k == 0), stop=(k == KH - 1),
                    )
                lgs = rpool.tile([P, E], f32, tag="lgs")
                nc.vector.tensor_tensor(out=lgs[:], in0=lgp[:], in1=rb_sb[:],
                                        op=OP.add)
                nc.sync.dma_start(lg_d[t * P:(t + 1) * P, :], lgs[:])
                # softmax over E
                m1 = rpool.tile([P, 1], f32, tag="m1")
                nc.vector.tensor_reduce(m1[:], lgs[:], axis=X, op=OP.max)
                sub = rpool.tile([P, E], f32, tag="sub")
                nc.vector.tensor_scalar(out=sub[:], in0=lgs[:], scalar1=m1[:, 0:1],
                                        scalar2=None, op0=OP.subtract)
                ex = rpool.tile([P, E], f32, tag="ex")
                nc.scalar.activation(ex[:], sub[:], AF.Exp)
                ssum = rpool.tile([P, 1], f32, tag="ssum")
                nc.vector.tensor_reduce(ssum[:], ex[:], axis=X, op=OP.add)
                rinv = rpool.tile([P, 1], f32, tag="rinv")
                nc.vector.reciprocal(rinv[:], ssum[:])
                smx = rpool.tile([P, E], f32, tag="smx")
                nc.vector.tensor_scalar(out=smx[:], in0=ex[:], scalar1=rinv[:, 0:1],
                                        scalar2=None, op0=OP.mult)
                nc.sync.dma_start(sm_d[t * P:(t + 1) * P, :], smx[:])
                # top-2 dispatch mask: g1 at argmax1, g2 at argmax2
                is1 = rpool.tile([P, E], f32, tag="is1")
                nc.vector.tensor_scalar(out=is1[:], in0=lgs[:], scalar1=m1[:, 0:1],
                                        scalar2=None, op0=OP.is_equal)
                mskd = rpool.tile([P, E], f32, tag="mskd")
                nc.vector.scalar_tensor_tensor(out=mskd[:], in0=is1[:], scalar=-1e30,
                                               in1=lgs[:], op0=OP.mult, op1=OP.add)
                m2 = rpool.tile([P, 1], f32, tag="m2")
                nc.vector.tensor_reduce(m2[:], mskd[:], axis=X, op=OP.max)
                is2 = rpool.tile([P, E], f32, tag="is2")
                nc.vector.tensor_scalar(out=is2[:], in0=mskd[:], scalar1=m2[:, 0:1],
                                        scalar2=None, op0=OP.is_equal)
                # g1 = sigmoid(m1-m2) as 1/(1+exp(m2-m1)): ACT only needs Exp
                d12 = rpool.tile([P, 1], f32, tag="d12")
                nc.vector.tensor_tensor(out=d12[:], in0=m2[:], in1=m1[:],
                                        op=OP.subtract)
                e12 = rpool.tile([P, 1], f32, tag="e12")
                nc.scalar.activation(e12[:], d12[:], AF.Exp)
                nc.vector.tensor_scalar(out=e12[:], in0=e12[:], scalar1=1.0,
                                        scalar2=None, op0=OP.add)
                g1 = rpool.tile([P, 1], f32, tag="g1")
                nc.vector.reciprocal(g1[:], e12[:])
                g2 = rpool.tile([P, 1], f32, tag="g2")
                nc.vector.tensor_scalar(out=g2[:], in0=g1[:], scalar1=-1.0,
                                        scalar2=1.0, op0=OP.mult, op1=OP.add)
                mk1 = rpool.tile([P, E], f32, tag="mk1")
                nc.vector.tensor_scalar(out=mk1[:], in0=is1[:], scalar1=g1[:, 0:1],
                                        scalar2=None, op0=OP.mult)
                mks = rpool.tile([P, E], f32, tag="mks")
                nc.vector.scalar_tensor_tensor(out=mks[:], in0=is2[:],
                                               scalar=g2[:, 0:1], in1=mk1[:],
                                               op0=OP.mult, op1=OP.add)
                nc.sync.dma_start(mk_d[t * P:(t + 1) * P, :], mks[:])

            def ffn_chunk(c, w1_sb, w2_sb, b1_sb, b2_sb, g_sb, xc=None):
                if xc is None:
                    xc = load_xc(c)
                # two half-tiles so next chunk's gelu can reuse half A while
                # this chunk's mm2 is still reading half B
                hTa = hpool.tile([P, KF // 2, CHUNK], mdt, tag="hTa")
                hTb = hpool.tile([P, KF // 2, CHUNK], mdt, tag="hTb")

                def hT(k):
                    return hTa[:, k, :] if k < KF // 2 else hTb[:, k - KF // 2, :]

                for m in range(KF):
                    hp = ps_h.tile([P, CHUNK], f32, tag="hp")
                    for k in range(KH):
                        nc.tensor.matmul(
                            hp[:], w1_sb[:, k, m * P:(m + 1) * P], xc[:, k, :],
                            start=(k == 0), stop=(k == KH - 1),
                        )
                    if gelu_mode == "lut":
                        nc.scalar.activation(hT(m), hp[:],
                                             AF.Gelu_apprx_tanh,
                                             bias=b1_sb[:, m:m + 1])
                    else:
                        # explicit tanh gelu (CoreSim-friendly)
                        xb = ypool.tile([P, CHUNK], f32, tag="xb")
                        nc.scalar.activation(xb[:], hp[:], AF.Identity,
                                             bias=b1_sb[:, m:m + 1])
                        sq = ypool.tile([P, CHUNK], f32, tag="sq")
                        nc.scalar.activation(sq[:], xb[:], AF.Square)
                        poly = ypool.tile([P, CHUNK], f32, tag="poly")
                        nc.vector.tensor_scalar(out=poly[:], in0=sq[:], scalar1=C1,
                                                scalar2=C0, op0=OP.mult, op1=OP.add)
                        nc.vector.tensor_tensor(out=poly[:], in0=poly[:], in1=xb[:],
                                                op=OP.mult)
                        th = ypool.tile([P, CHUNK], f32, tag="th")
                        nc.scalar.activation(th[:], poly[:], AF.Tanh)
                        nc.vector.tensor_scalar(out=th[:], in0=th[:], scalar1=1.0,
                                                scalar2=0.5, op0=OP.add, op1=OP.mult)
                        nc.vector.tensor_tensor(out=hT(m), in0=th[:],
                                                in1=xb[:], op=OP.mult)
                for tt in range(CHUNK // P):
                    yp = ps_y.tile([P, H], f32, tag="yp")
                    for k in range(KF):
                        hTk = hT(k)[:, tt * P:(tt + 1) * P]
                        nc.tensor.matmul(yp[:, 0:512], hTk,
                                         w2_sb[:, k, 0:512],
                                         start=(k == 0), stop=(k == KF - 1))
                        nc.tensor.matmul(yp[:, 512:H], hTk,
                                         w2_sb[:, k, 512:H],
                                         start=(k == 0), stop=(k == KF - 1))
                    ys = ypool.tile([P, H], f32, tag="ys")
                    j = c * (CHUNK // P) + tt
                    if b2_zero:
                        nc.vector.tensor_scalar(out=ys[:], in0=yp[:],
                                                scalar1=g_sb[:, j:j + 1],
                                                scalar2=None, op0=OP.mult)
                    else:
                        nc.vector.tensor_tensor(out=ys[:], in0=yp[:],
                                                in1=b2_sb[:], op=OP.add)
                        nc.vector.tensor_scalar(out=ys[:], in0=ys[:],
                                                scalar1=g_sb[:, j:j + 1],
                                                scalar2=None, op0=OP.mult)
                    nc.sync.dma_start(y_d[j * P:(j + 1) * P, :], ys[:])

            for _rep in range(reps):
                nts = ntr if "router" in sections else 0
                ncs = nch if "ffn" in sections else 0

                # --- DMA priority order: tiny consts + first working set
                # first, then the big weight streams (in the order the PE
                # consumes them), so the PE starts within ~10us instead of
                # waiting for the full 19MB weight load.
                b1_sb = cpool.tile([P, KF], f32, tag="b1")
                nc.sync.dma_start(b1_sb[:], b1_d[:])
                b2_sb = None
                if not b2_zero:
                    b2_sb = cpool.tile([P, H], f32, tag="b2")
                    nc.sync.dma_start(b2_sb[:], b2_d[:])
                g_sb = cpool.tile([P, C // P], f32, tag="g")
                nc.sync.dma_start(g_sb[:], g_d[:])
                rwt_sb = cpool.tile([P, KH, E], f32, tag="rwt")
                nc.sync.dma_start(rwt_sb[:], rwt_d[:])
                rb_sb = cpool.tile([P, E], f32, tag="rb")
                nc.sync.dma_start(rb_sb[:], rb_d[:])
                xr0 = load_xr(0) if nts else None
                xc0 = load_xc(0) if ncs else None

                # w1 streamed in column blocks (mm1 consumes columns in
                # order); w2 streamed in k blocks (mm2 consumption order)
                w1_sb = wpool.tile([P, KH, F], mdt, tag="w1")
                nc.sync.dma_start(w1_sb[:, :, 0:128], w1_d[:, :, 0:128])
                nc.sync.dma_start(w1_sb[:, :, 128:384], w1_d[:, :, 128:384])
                for ms in range(384, F, 384):
                    nc.sync.dma_start(w1_sb[:, :, ms:ms + 384],
                                      w1_d[:, :, ms:ms + 384])
                w2_sb = wpool.tile([P, KF, H], mdt, tag="w2")
                for k in range(0, KF, 2):
                    nc.sync.dma_start(w2_sb[:, k:k + 2, :], w2_d[:, k:k + 2, :])

                # interleave: one router token-tile between FFN chunks so the
                # router's vector/scalar chain hides under the FFN matmuls
                for c in range(max(ncs, nts)):
                    if c < nts:
                        router_tile(c, rwt_sb, rb_sb, xr_sb=xr0 if c == 0 else None)
                    if c < ncs:
                        ffn_chunk(c, w1_sb, w2_sb, b1_sb, b2_sb, g_sb,
                                  xc=xc0 if c == 0 else None)

    nc.compile()
    return nc


def _xT_blocks(xmat, kblocks):
    """[N, kblocks*P] row-major -> [P, kblocks, N] (transposed block layout)."""
    n = xmat.shape[0]
    return np.ascontiguousarray(
        xmat.T.reshape(kblocks, P, n).transpose(1, 0, 2)
    )


def host_route(x, router_w, router_b):
    """numpy replica of the routing decision (indices + gates for dispatch)."""
    logits = (x @ router_w + router_b).astype(np.float32)
    idx1 = np.argmax(logits, axis=1)
    rows = np.arange(x.shape[0])
    masked = logits.copy()
    masked[rows, idx1] = -np.inf
    idx2 = np.argmax(masked, axis=1)
    v1 = logits[rows, idx1]
    v2 = logits[rows, idx2]
    g1 = (1.0 / (1.0 + np.exp((v2 - v1).astype(np.float64)))).astype(np.float32)
    g2 = np.float32(1.0) - g1
    return logits, idx1, idx2, g1, g2


def kernel(hidden_states, router_w, router_b, w1, b1, w2, b2):
    hidden_states = np.asarray(hidden_states, dtype=np.float32)
    router_w = np.asarray(router_w, dtype=np.float32)
    router_b = np.asarray(router_b, dtype=np.float32)
    w1 = np.asarray(w1, dtype=np.float32)
    b1 = np.asarray(b1, dtype=np.float32)
    w2 = np.asarray(w2, dtype=np.float32)
    b2 = np.asarray(b2, dtype=np.float32)

    try:
        import jax
        jax.config.update("jax_compilation_cache_dir", "/tmp/jax_cache")
        jax.config.update("jax_persistent_cache_min_compile_time_secs", 1.0)
    except Exception:
        pass

    B, S, Hd = hidden_states.shape
    assert Hd == H
    T = B * S
    assert T % N_CORES == 0
    TS = T // N_CORES
    x = np.ascontiguousarray(hidden_states.reshape(T, H))

    # ---- host routing decision (for the expert-parallel gather only) ----
    logits_h, idx1, idx2, g1, g2 = host_route(x, router_w, router_b)

    tok_lists, gate_lists = [], []
    for e in range(E):
        s1 = idx1 == e
        s2 = idx2 == e
        tok = np.nonzero(s1 | s2)[0]
        gate = np.where(s1[tok], g1[tok], g2[tok]).astype(np.float32)
        tok_lists.append(tok)
        gate_lists.append(gate)
    maxcnt = max(len(t) for t in tok_lists)
    C = int(-(-maxcnt // CHUNK) * CHUNK)

    reps = int(os.environ.get("MOE_REPS", "1"))
    mm_dtype = os.environ.get("MOE_MM_DTYPE", "float32r")
    gelu_mode = os.environ.get("MOE_GELU", "lut")
    b2_zero = bool(np.all(b2 == 0))
    key = (C, TS, reps, mm_dtype, gelu_mode, b2_zero)
    if key not in _PROGRAM_CACHE:
        _PROGRAM_CACHE[key] = build_program(
            C, TS, reps=reps, mm_dtype_name=mm_dtype, gelu_mode=gelu_mode,
            b2_zero=b2_zero)
    nc = _PROGRAM_CACHE[key]

    import ml_dtypes
    np_mdt = np.float32 if mm_dtype == "float32r" else ml_dtypes.bfloat16

    rwt_arr = np.ascontiguousarray(
        router_w.reshape(KH, P, E).transpose(1, 0, 2))
    rb_arr = np.ascontiguousarray(np.broadcast_to(router_b, (P, E)))

    in_maps = []
    for e in range(E):
        tok = tok_lists[e]
        n = len(tok)
        xg = np.zeros((C, H), dtype=np.float32)
        xg[:n] = x[tok]
        gates = np.zeros((C,), dtype=np.float32)
        gates[:n] = gate_lists[e]
        xs = x[e * TS:(e + 1) * TS]
        im = {
            "xT_ffn": _xT_blocks(xg, KH).astype(np_mdt),
            "w1": np.ascontiguousarray(
                w1[e].reshape(KH, P, F).transpose(1, 0, 2)).astype(np_mdt),
            "w2": np.ascontiguousarray(
                w2[e].reshape(KF, P, H).transpose(1, 0, 2)).astype(np_mdt),
            "b1": np.ascontiguousarray(b1[e].reshape(KF, P).T),
            "gates": np.ascontiguousarray(gates.reshape(C // P, P).T),
            "x_r": _xT_blocks(xs, KH),
            "rwght": rwt_arr,
            "rbias": rb_arr,
        }
        if not b2_zero:
            im["b2"] = np.ascontiguousarray(np.broadcast_to(b2[e], (P, H)))
        in_maps.append(im)

    from concourse.bass_utils import run_bass_kernel_spmd
    res = run_bass_kernel_spmd(nc, in_maps, core_ids=list(range(N_CORES)))
    results = res.results

    # ---- gather / unshard ----
    combined = np.zeros((T, H), dtype=np.float32)
    for e in range(E):
        tok = tok_lists[e]
        combined[tok] += results[e]["y"][:len(tok)]

    logits = np.concatenate([results[i]["logits"] for i in range(N_CORES)], axis=0)
    softmax = np.concatenate([results[i]["softmax"] for i in range(N_CORES)], axis=0)
    mask = np.concatenate([results[i]["mask"] for i in range(N_CORES)], axis=0)
    mean_prob = softmax.mean(axis=0, dtype=np.float64)
    tokens_per_expert = mask.sum(axis=0, dtype=np.float64) / T
    aux = np.float32(E * np.dot(mean_prob, tokens_per_expert))

    return (
        combined.reshape(B, S, H),
        softmax.reshape(B, S, E),
        mask.reshape(B, S, E),
        np.float32(aux),
        logits.reshape(B, S, E),
    )


# revision 30
# speedup vs baseline: 1474.1801x; 1.0052x over previous
"""Trainium2 Bass kernel for an 8-expert top-2 MoE layer (768 hidden, 3072 FFN).

Strategy (expert-parallel over 8 NeuronCores):
  - Each core owns one expert's FFN weights (w1[e], b1[e], w2[e], b2[e]).
  - The host computes routing indices (which tokens go to which expert) and
    gathers/pads each expert's tokens to a fixed capacity C; the device
    computes (gelu(x@w1+b1)@w2+b2)*gate for those tokens.
  - The router itself (logits, softmax, top-2 dispatch mask) is ALSO computed
    on device, data-parallel: core i handles tokens [i*T/8, (i+1)*T/8).
    Router tiles are interleaved between FFN chunks so their vector/scalar
    work hides under the FFN matmuls.
  - The host scatters the per-expert outputs back (each token receives
    exactly TOP_K=2 contributions) and reduces the aux loss from the
    returned routing outputs.

All matmuls use float32r (fp32 data at ~1 cycle/row on the PE when the
moving free dim >= 256).
"""

import os
import numpy as np

P = 128
E = 8
H = 768
F = 3072
KH = H // P      # 6
KF = F // P      # 24
CHUNK = 256      # tokens per mm1 moving block (>=256 keeps float32r full-rate)
N_CORES = 8

_PROGRAM_CACHE = {}


def build_program(C, TS, reps=1, mm_dtype_name="float32r", gelu_mode="lut",
                  b2_zero=True, sections=("router", "ffn")):
    """Build + compile the per-core SPMD Bass program.

    C:  padded per-expert token capacity (multiple of CHUNK)
    TS: tokens per core for the router section (T / 8)
    reps: repeat the whole body (for wall-clock delta timing); outputs are
          rewritten identically each rep.
    """
    import concourse.mybir as mybir
    import concourse.tile as tile
    from concourse import bacc

    f32 = mybir.dt.float32
    mdt = getattr(mybir.dt, mm_dtype_name)
    AF = mybir.ActivationFunctionType
    OP = mybir.AluOpType
    X = mybir.AxisListType.X

    assert C % CHUNK == 0 and TS % P == 0
    nch = C // CHUNK
    ntr = TS // P

    nc = bacc.Bacc(None, target_bir_lowering=False)

    xT_ffn = nc.dram_tensor("xT_ffn", [P, KH, C], mdt, kind="ExternalInput")
    w1_d = nc.dram_tensor("w1", [P, KH, F], mdt, kind="ExternalInput")
    w2_d = nc.dram_tensor("w2", [P, KF, H], mdt, kind="ExternalInput")
    b1_d = nc.dram_tensor("b1", [P, KF], f32, kind="ExternalInput")
    b2_d = None
    if not b2_zero:
        b2_d = nc.dram_tensor("b2", [P, H], f32, kind="ExternalInput")
    g_d = nc.dram_tensor("gates", [P, C // P], f32, kind="ExternalInput")
    xr_d = nc.dram_tensor("x_r", [P, KH, TS], f32, kind="ExternalInput")
    rwt_d = nc.dram_tensor("rwght", [P, KH, E], f32, kind="ExternalInput")
    rb_d = nc.dram_tensor("rbias", [P, E], f32, kind="ExternalInput")

    y_d = nc.dram_tensor("y", [C, H], f32, kind="ExternalOutput")
    lg_d = nc.dram_tensor("logits", [TS, E], f32, kind="ExternalOutput")
    sm_d = nc.dram_tensor("softmax", [TS, E], f32, kind="ExternalOutput")
    mk_d = nc.dram_tensor("mask", [TS, E], f32, kind="ExternalOutput")

    # gelu-tanh constants: 0.5*x*(1+tanh(c0*x + c1*x^3))
    C0 = 0.7978845608028654
    C1 = C0 * 0.044715

    with tile.TileContext(nc) as tc:
        with (
            tc.tile_pool(name="wpool", bufs=1) as wpool,
            tc.tile_pool(name="cpool", bufs=1) as cpool,
            tc.tile_pool(name="xpool", bufs=2) as xpool,
            tc.tile_pool(name="hpool", bufs=3 if gelu_mode == "lut" else 2) as hpool,
            tc.tile_pool(name="ypool", bufs=2) as ypool,
            tc.tile_pool(name="rpool", bufs=2) as rpool,
            tc.tile_pool(name="ps_h", bufs=3, space="PSUM") as ps_h,
            tc.tile_pool(name="ps_y", bufs=2, space="PSUM") as ps_y,
            tc.tile_pool(name="ps_r", bufs=1, space="PSUM") as ps_r,
        ):
            def load_xr(t):
                xr_sb = rpool.tile([P, KH, P], f32, tag="xr")
                nc.sync.dma_start(xr_sb[:], xr_d[:, :, t * P:(t + 1) * P])
                return xr_sb

            def load_xc(c):
                xc = xpool.tile([P, KH, CHUNK], mdt, tag="xc")
                nc.sync.dma_start(xc[:], xT_ffn[:, :, c * CHUNK:(c + 1) * CHUNK])
                return xc

            def router_tile(t, rwt_sb, rb_sb, xr_sb=None):
                if xr_sb is None:
                    xr_sb = load_xr(t)
                lgp = ps_r.tile([P, E], f32, tag="lg")
                for k in range(KH):
                    nc.tensor.matmul(
                        lgp[:], xr_sb[:, k, :], rwt_sb[:, k, :],
                        start=(k == 0), stop=(k == KH - 1),
                    )
                lgs = rpool.tile([P, E], f32, tag="lgs")
                nc.vector.tensor_tensor(out=lgs[:], in0=lgp[:], in1=rb_sb[:],
                                        op=OP.add)
                nc.sync.dma_start(lg_d[t * P:(t + 1) * P, :], lgs[:])
                # softmax over E
                m1 = rpool.tile([P, 1], f32, tag="m1")
                nc.vector.tensor_reduce(m1[:], lgs[:], axis=X, op=OP.max)
                sub = rpool.tile([P, E], f32, tag="sub")
                nc.vector.tensor_scalar(out=sub[:], in0=lgs[:], scalar1=m1[:, 0:1],
                                        scalar2=None, op0=OP.subtract)
                ex = rpool.tile([P, E], f32, tag="ex")
                nc.scalar.activation(ex[:], sub[:], AF.Exp)
                ssum = rpool.tile([P, 1], f32, tag="ssum")
                nc.vector.tensor_reduce(ssum[:], ex[:], axis=X, op=OP.add)
                rinv = rpool.tile([P, 1], f32, tag="rinv")
                nc.vector.reciprocal(rinv[:], ssum[:])
                smx = rpool.tile([P, E], f32, tag="smx")
                nc.vector.tensor_scalar(out=smx[:], in0=ex[:], scalar1=rinv[:, 0:1],
                                        scalar2=None, op0=OP.mult)
                nc.sync.dma_start(sm_d[t * P:(t + 1) * P, :], smx[:])
                # top-2 dispatch mask: g1 at argmax1, g2 at argmax2
                is1 = rpool.tile([P, E], f32, tag="is1")
                nc.vector.tensor_scalar(out=is1[:], in0=lgs[:], scalar1=m1[:, 0:1],
                                        scalar2=None, op0=OP.is_equal)
                mskd = rpool.tile([P, E], f32, tag="mskd")
                nc.vector.scalar_tensor_tensor(out=mskd[:], in0=is1[:], scalar=-1e30,
                                               in1=lgs[:], op0=OP.mult, op1=OP.add)
                m2 = rpool.tile([P, 1], f32, tag="m2")
                nc.vector.tensor_reduce(m2[:], mskd[:], axis=X, op=OP.max)
                is2 = rpool.tile([P, E], f32, tag="is2")
                nc.vector.tensor_scalar(out=is2[:], in0=mskd[:], scalar1=m2[:, 0:1],
                                        scalar2=None, op0=OP.is_equal)
                # g1 = sigmoid(m1-m2) as 1/(1+exp(m2-m1)): ACT only needs Exp
                d12 = rpool.tile([P, 1], f32, tag="d12")
                nc.vector.tensor_tensor(out=d12[:], in0=m2[:], in1=m1[:],
                                        op=OP.subtract)
                e12 = rpool.tile([P, 1], f32, tag="e12")
                nc.scalar.activation(e12[:], d12[:], AF.Exp)
                nc.vector.tensor_scalar(out=e12[:], in0=e12[:], scalar1=1.0,
                                        scalar2=None, op0=OP.add)
                g1 = rpool.tile([P, 1], f32, tag="g1")
                nc.vector.reciprocal(g1[:], e12[:])
                g2 = rpool.tile([P, 1], f32, tag="g2")
                nc.vector.tensor_scalar(out=g2[:], in0=g1[:], scalar1=-1.0,
                                        scalar2=1.0, op0=OP.mult, op1=OP.add)
                mk1 = rpool.tile([P, E], f32, tag="mk1")
                nc.vector.tensor_scalar(out=mk1[:], in0=is1[:], scalar1=g1[:, 0:1],
                                        scalar2=None, op0=OP.mult)
                mks = rpool.tile([P, E], f32, tag="mks")
                nc.vector.scalar_tensor_tensor(out=mks[:], in0=is2[:],
                                               scalar=g2[:, 0:1], in1=mk1[:],
                                               op0=OP.mult, op1=OP.add)
                nc.sync.dma_start(mk_d[t * P:(t + 1) * P, :], mks[:])

            def mm1_half(c, half, xc, halves, w1_sb, b1_sb):
                # computes hT half-tile (12 of 24 k-blocks) for chunk c
                ht = hpool.tile([P, KF // 2, CHUNK], mdt, tag="hT")
                halves[(c, half)] = ht
                for mi in range(KF // 2):
                    m = half * (KF // 2) + mi
                    hp = ps_h.tile([P, CHUNK], f32, tag="hp")
                    for k in range(KH):
                        nc.tensor.matmul(
                            hp[:], w1_sb[:, k, m * P:(m + 1) * P], xc[:, k, :],
                            start=(k == 0), stop=(k == KH - 1),
                        )
                    if gelu_mode == "lut":
                        nc.scalar.activation(ht[:, mi, :], hp[:],
                                             AF.Gelu_apprx_tanh,
                                             bias=b1_sb[:, m:m + 1])
                    else:
                        # explicit tanh gelu (CoreSim-friendly)
                        xb = ypool.tile([P, CHUNK], f32, tag="xb")
                        nc.scalar.activation(xb[:], hp[:], AF.Identity,
                                             bias=b1_sb[:, m:m + 1])
                        sq = ypool.tile([P, CHUNK], f32, tag="sq")
                        nc.scalar.activation(sq[:], xb[:], AF.Square)
                        poly = ypool.tile([P, CHUNK], f32, tag="poly")
                        nc.vector.tensor_scalar(out=poly[:], in0=sq[:], scalar1=C1,
                                                scalar2=C0, op0=OP.mult, op1=OP.add)
                        nc.vector.tensor_tensor(out=poly[:], in0=poly[:], in1=xb[:],
                                                op=OP.mult)
                        th = ypool.tile([P, CHUNK], f32, tag="th")
                        nc.scalar.activation(th[:], poly[:], AF.Tanh)
                        nc.vector.tensor_scalar(out=th[:], in0=th[:], scalar1=1.0,
                                                scalar2=0.5, op0=OP.add, op1=OP.mult)
                        nc.vector.tensor_tensor(out=ht[:, mi, :], in0=th[:],
                                                in1=xb[:], op=OP.mult)

            def mm2_chunk(c, halves, w2_sb, b2_sb, g_sb):
                hTa = halves.pop((c, 0))
                hTb = halves.pop((c, 1))
                for tt in range(CHUNK // P):
                    yp = ps_y.tile([P, H], f32, tag="yp")
                    for k in range(KF):
                        src_t = hTa if k < KF // 2 else hTb
                        hTk = src_t[:, k % (KF // 2), tt * P:(tt + 1) * P]
                        nc.tensor.matmul(yp[:, 0:512], hTk,
                                         w2_sb[:, k, 0:512],
                                         start=(k == 0), stop=(k == KF - 1))
                        nc.tensor.matmul(yp[:, 512:H], hTk,
                                         w2_sb[:, k, 512:H],
                                         start=(k == 0), stop=(k == KF - 1))
                    ys = ypool.tile([P, H], f32, tag="ys")
                    j = c * (CHUNK // P) + tt
                    if b2_zero:
                        nc.vector.tensor_scalar(out=ys[:], in0=yp[:],
                                                scalar1=g_sb[:, j:j + 1],
                                                scalar2=None, op0=OP.mult)
                    else:
                        nc.vector.tensor_tensor(out=ys[:], in0=yp[:],
                                                in1=b2_sb[:], op=OP.add)
                        nc.vector.tensor_scalar(out=ys[:], in0=ys[:],
                                                scalar1=g_sb[:, j:j + 1],
                                                scalar2=None, op0=OP.mult)
                    nc.sync.dma_start(y_d[j * P:(j + 1) * P, :], ys[:])

            for _rep in range(reps):
                nts = ntr if "router" in sections else 0
                ncs = nch if "ffn" in sections else 0

                # --- DMA priority order: tiny consts + first working set
                # first, then the big weight streams (in the order the PE
                # consumes them), so the PE starts within ~10us instead of
                # waiting for the full 19MB weight load.
                rwt_sb = cpool.tile([P, KH, E], f32, tag="rwt")
                nc.sync.dma_start(rwt_sb[:], rwt_d[:])
                rb_sb = cpool.tile([P, E], f32, tag="rb")
                nc.sync.dma_start(rb_sb[:], rb_d[:])
                xr0 = load_xr(0) if nts else None
                xc0 = load_xc(0) if ncs else None

                # w1 streamed in column blocks (mm1 consumes columns in
                # order); w2 streamed in k blocks (mm2 consumption order).
                # b1/gates are not needed until the first gelu/eviction, so
                # they queue after the first w1 block.
                w1_sb = wpool.tile([P, KH, F], mdt, tag="w1")
                nc.sync.dma_start(w1_sb[:, :, 0:128], w1_d[:, :, 0:128])
                b1_sb = cpool.tile([P, KF], f32, tag="b1")
                nc.sync.dma_start(b1_sb[:], b1_d[:])
                b2_sb = None
                if not b2_zero:
                    b2_sb = cpool.tile([P, H], f32, tag="b2")
                    nc.sync.dma_start(b2_sb[:], b2_d[:])
                g_sb = cpool.tile([P, C // P], f32, tag="g")
                nc.sync.dma_start(g_sb[:], g_d[:])
                nc.sync.dma_start(w1_sb[:, :, 128:384], w1_d[:, :, 128:384])
                for ms in range(384, F, 384):
                    nc.sync.dma_start(w1_sb[:, :, ms:ms + 384],
                                      w1_d[:, :, ms:ms + 384])
                w2_sb = wpool.tile([P, KF, H], mdt, tag="w2")
                for k in range(0, KF, 2):
                    nc.sync.dma_start(w2_sb[:, k:k + 2, :], w2_d[:, k:k + 2, :])

                # software pipeline: mm1 halves of chunk c+1/c+2 are emitted
                # around mm2(c) so the PE has mm1 work to run while mm2
                # trails the streaming w2 during startup. Router tiles are
                # interleaved one per chunk so their vector/scalar chains
                # hide under the FFN matmuls.
                halves = {}
                xcs = {0: xc0}
                if ncs:
                    if nts:
                        router_tile(0, rwt_sb, rb_sb, xr_sb=xr0)
                    mm1_half(0, 0, xcs[0], halves, w1_sb, b1_sb)
                    mm1_half(0, 1, xcs[0], halves, w1_sb, b1_sb)
                    if ncs > 1:
                        xcs[1] = load_xc(1)
                        mm1_half(1, 0, xcs[1], halves, w1_sb, b1_sb)
                    for c in range(ncs):
                        if 1 + c < nts:
                            router_tile(1 + c, rwt_sb, rb_sb)
                        mm2_chunk(c, halves, w2_sb, b2_sb, g_sb)
                        if c + 1 < ncs:
                            mm1_half(c + 1, 1, xcs[c + 1], halves, w1_sb, b1_sb)
                        if c + 2 < ncs:
                            xcs[c + 2] = load_xc(c + 2)
                            mm1_half(c + 2, 0, xcs[c + 2], halves, w1_sb, b1_sb)
                        xcs.pop(c, None)
                    for t in range(ncs + 1, nts):
                        router_tile(t, rwt_sb, rb_sb)
                else:
                    for t in range(nts):
                        router_tile(t, rwt_sb, rb_sb, xr_sb=xr0 if t == 0 else None)

    nc.compile()
    return nc


def _xT_blocks(xmat, kblocks):
    """[N, kblocks*P] row-major -> [P, kblocks, N] (transposed block layout)."""
    n = xmat.shape[0]
    return np.ascontiguousarray(
        xmat.T.reshape(kblocks, P, n).transpose(1, 0, 2)
    )


def host_route(x, router_w, router_b):
    """numpy replica of the routing decision (indices + gates for dispatch)."""
    logits = (x @ router_w + router_b).astype(np.float32)
    idx1 = np.argmax(logits, axis=1)
    rows = np.arange(x.shape[0])
    masked = logits.copy()
    masked[rows, idx1] = -np.inf
    idx2 = np.argmax(masked, axis=1)
    v1 = logits[rows, idx1]
    v2 = logits[rows, idx2]
    g1 = (1.0 / (1.0 + np.exp((v2 - v1).astype(np.float64)))).astype(np.float32)
    g2 = np.float32(1.0) - g1
    return logits, idx1, idx2, g1, g2


def kernel(hidden_states, router_w, router_b, w1, b1, w2, b2):
    hidden_states = np.asarray(hidden_states, dtype=np.float32)
    router_w = np.asarray(router_w, dtype=np.float32)
    router_b = np.asarray(router_b, dtype=np.float32)
    w1 = np.asarray(w1, dtype=np.float32)
    b1 = np.asarray(b1, dtype=np.float32)
    w2 = np.asarray(w2, dtype=np.float32)
    b2 = np.asarray(b2, dtype=np.float32)

    try:
        import jax
        jax.config.update("jax_compilation_cache_dir", "/tmp/jax_cache")
        jax.config.update("jax_persistent_cache_min_compile_time_secs", 1.0)
    except Exception:
        pass

    B, S, Hd = hidden_states.shape
    assert Hd == H
    T = B * S
    assert T % N_CORES == 0
    TS = T // N_CORES
    x = np.ascontiguousarray(hidden_states.reshape(T, H))

    # ---- host routing decision (for the expert-parallel gather only) ----
    logits_h, idx1, idx2, g1, g2 = host_route(x, router_w, router_b)

    tok_lists, gate_lists = [], []
    for e in range(E):
        s1 = idx1 == e
        s2 = idx2 == e
        tok = np.nonzero(s1 | s2)[0]
        gate = np.where(s1[tok], g1[tok], g2[tok]).astype(np.float32)
        tok_lists.append(tok)
        gate_lists.append(gate)
    maxcnt = max(len(t) for t in tok_lists)
    C = int(-(-maxcnt // CHUNK) * CHUNK)

    reps = int(os.environ.get("MOE_REPS", "1"))
    mm_dtype = os.environ.get("MOE_MM_DTYPE", "float32r")
    gelu_mode = os.environ.get("MOE_GELU", "lut")
    b2_zero = bool(np.all(b2 == 0))
    key = (C, TS, reps, mm_dtype, gelu_mode, b2_zero)
    if key not in _PROGRAM_CACHE:
        _PROGRAM_CACHE[key] = build_program(
            C, TS, reps=reps, mm_dtype_name=mm_dtype, gelu_mode=gelu_mode,
            b2_zero=b2_zero)
    nc = _PROGRAM_CACHE[key]

    import ml_dtypes
    np_mdt = np.float32 if mm_dtype == "float32r" else ml_dtypes.bfloat16

    rwt_arr = np.ascontiguousarray(
        router_w.reshape(KH, P, E).transpose(1, 0, 2))
    rb_arr = np.ascontiguousarray(np.broadcast_to(router_b, (P, E)))

    in_maps = []
    for e in range(E):
        tok = tok_lists[e]
        n = len(tok)
        xg = np.zeros((C, H), dtype=np.float32)
        xg[:n] = x[tok]
        gates = np.zeros((C,), dtype=np.float32)
        gates[:n] = gate_lists[e]
        xs = x[e * TS:(e + 1) * TS]
        im = {
            "xT_ffn": _xT_blocks(xg, KH).astype(np_mdt),
            "w1": np.ascontiguousarray(
                w1[e].reshape(KH, P, F).transpose(1, 0, 2)).astype(np_mdt),
            "w2": np.ascontiguousarray(
                w2[e].reshape(KF, P, H).transpose(1, 0, 2)).astype(np_mdt),
            "b1": np.ascontiguousarray(b1[e].reshape(KF, P).T),
            "gates": np.ascontiguousarray(gates.reshape(C // P, P).T),
            "x_r": _xT_blocks(xs, KH),
            "rwght": rwt_arr,
            "rbias": rb_arr,
        }
        if not b2_zero:
            im["b2"] = np.ascontiguousarray(np.broadcast_to(b2[e], (P, H)))
        in_maps.append(im)

    from concourse.bass_utils import run_bass_kernel_spmd
    res = run_bass_kernel_spmd(nc, in_maps, core_ids=list(range(N_CORES)))
    results = res.results

    # ---- gather / unshard ----
    combined = np.zeros((T, H), dtype=np.float32)
    for e in range(E):
        tok = tok_lists[e]
        combined[tok] += results[e]["y"][:len(tok)]

    logits = np.concatenate([results[i]["logits"] for i in range(N_CORES)], axis=0)
    softmax = np.concatenate([results[i]["softmax"] for i in range(N_CORES)], axis=0)
    mask = np.concatenate([results[i]["mask"] for i in range(N_CORES)], axis=0)
    mean_prob = softmax.mean(axis=0, dtype=np.float64)
    tokens_per_expert = mask.sum(axis=0, dtype=np.float64) / T
    aux = np.float32(E * np.dot(mean_prob, tokens_per_expert))

    return (
        combined.reshape(B, S, H),
        softmax.reshape(B, S, E),
        mask.reshape(B, S, E),
        np.float32(aux),
        logits.reshape(B, S, E),
    )


# revision 31
# speedup vs baseline: 1475.3411x; 1.0008x over previous
"""Trainium2 Bass kernel for an 8-expert top-2 MoE layer (768 hidden, 3072 FFN).

Strategy (expert-parallel over 8 NeuronCores):
  - Each core owns one expert's FFN weights (w1[e], b1[e], w2[e], b2[e]).
  - The host computes routing indices (which tokens go to which expert) and
    gathers/pads each expert's tokens to a fixed capacity C; the device
    computes (gelu(x@w1+b1)@w2+b2)*gate for those tokens.
  - The router itself (logits, softmax, top-2 dispatch mask) is ALSO computed
    on device, data-parallel: core i handles tokens [i*T/8, (i+1)*T/8).
    Router tiles are interleaved between FFN chunks so their vector/scalar
    work hides under the FFN matmuls.
  - The host scatters the per-expert outputs back (each token receives
    exactly TOP_K=2 contributions) and reduces the aux loss from the
    returned routing outputs.

All matmuls use float32r (fp32 data at ~1 cycle/row on the PE when the
moving free dim >= 256).
"""

import os
import numpy as np

P = 128
E = 8
H = 768
F = 3072
KH = H // P      # 6
KF = F // P      # 24
CHUNK = 256      # tokens per mm1 moving block (>=256 keeps float32r full-rate)
N_CORES = 8

_PROGRAM_CACHE = {}


def build_program(C, TS, reps=1, mm_dtype_name="float32r", gelu_mode="lut",
                  b2_zero=True, sections=("router", "ffn")):
    """Build + compile the per-core SPMD Bass program.

    C:  padded per-expert token capacity (multiple of CHUNK)
    TS: tokens per core for the router section (T / 8)
    reps: repeat the whole body (for wall-clock delta timing); outputs are
          rewritten identically each rep.
    """
    import concourse.mybir as mybir
    import concourse.tile as tile
    from concourse import bacc

    f32 = mybir.dt.float32
    mdt = getattr(mybir.dt, mm_dtype_name)
    AF = mybir.ActivationFunctionType
    OP = mybir.AluOpType
    X = mybir.AxisListType.X

    assert C % CHUNK == 0 and TS % P == 0
    nch = C // CHUNK
    ntr = TS // P

    nc = bacc.Bacc(None, target_bir_lowering=False)

    xT_ffn = nc.dram_tensor("xT_ffn", [P, KH, C], mdt, kind="ExternalInput")
    w1_d = nc.dram_tensor("w1", [P, KH, F], mdt, kind="ExternalInput")
    w2_d = nc.dram_tensor("w2", [P, KF, H], mdt, kind="ExternalInput")
    b1_d = nc.dram_tensor("b1", [P, KF], f32, kind="ExternalInput")
    b2_d = None
    if not b2_zero:
        b2_d = nc.dram_tensor("b2", [P, H], f32, kind="ExternalInput")
    g_d = nc.dram_tensor("gates", [P, C // P], f32, kind="ExternalInput")
    xr_d = nc.dram_tensor("x_r", [P, KH, TS], f32, kind="ExternalInput")
    rwt_d = nc.dram_tensor("rwght", [P, KH, E], f32, kind="ExternalInput")
    rb_d = nc.dram_tensor("rbias", [P, E], f32, kind="ExternalInput")

    y_d = nc.dram_tensor("y", [C, H], f32, kind="ExternalOutput")
    lg_d = nc.dram_tensor("logits", [TS, E], f32, kind="ExternalOutput")
    sm_d = nc.dram_tensor("softmax", [TS, E], f32, kind="ExternalOutput")
    mk_d = nc.dram_tensor("mask", [TS, E], f32, kind="ExternalOutput")

    # gelu-tanh constants: 0.5*x*(1+tanh(c0*x + c1*x^3))
    C0 = 0.7978845608028654
    C1 = C0 * 0.044715

    with tile.TileContext(nc) as tc:
        with (
            tc.tile_pool(name="wpool", bufs=1) as wpool,
            tc.tile_pool(name="cpool", bufs=1) as cpool,
            tc.tile_pool(name="xpool", bufs=2) as xpool,
            tc.tile_pool(name="hpool", bufs=3 if gelu_mode == "lut" else 2) as hpool,
            tc.tile_pool(name="ypool", bufs=2) as ypool,
            tc.tile_pool(name="rpool", bufs=2) as rpool,
            tc.tile_pool(name="ps_h", bufs=3, space="PSUM") as ps_h,
            tc.tile_pool(name="ps_y", bufs=2, space="PSUM") as ps_y,
            tc.tile_pool(name="ps_r", bufs=1, space="PSUM") as ps_r,
        ):
            def load_xr(t):
                xr_sb = rpool.tile([P, KH, P], f32, tag="xr")
                nc.sync.dma_start(xr_sb[:], xr_d[:, :, t * P:(t + 1) * P])
                return xr_sb

            def load_xc(c):
                xc = xpool.tile([P, KH, CHUNK], mdt, tag="xc")
                nc.sync.dma_start(xc[:], xT_ffn[:, :, c * CHUNK:(c + 1) * CHUNK])
                return xc

            def router_tile(t, rwt_sb, rb_sb, xr_sb=None):
                if xr_sb is None:
                    xr_sb = load_xr(t)
                lgp = ps_r.tile([P, E], f32, tag="lg")
                for k in range(KH):
                    nc.tensor.matmul(
                        lgp[:], xr_sb[:, k, :], rwt_sb[:, k, :],
                        start=(k == 0), stop=(k == KH - 1),
                    )
                lgs = rpool.tile([P, E], f32, tag="lgs")
                nc.vector.tensor_tensor(out=lgs[:], in0=lgp[:], in1=rb_sb[:],
                                        op=OP.add)
                nc.sync.dma_start(lg_d[t * P:(t + 1) * P, :], lgs[:])
                # softmax over E
                m1 = rpool.tile([P, 1], f32, tag="m1")
                nc.vector.tensor_reduce(m1[:], lgs[:], axis=X, op=OP.max)
                sub = rpool.tile([P, E], f32, tag="sub")
                nc.vector.tensor_scalar(out=sub[:], in0=lgs[:], scalar1=m1[:, 0:1],
                                        scalar2=None, op0=OP.subtract)
                ex = rpool.tile([P, E], f32, tag="ex")
                nc.scalar.activation(ex[:], sub[:], AF.Exp)
                ssum = rpool.tile([P, 1], f32, tag="ssum")
                nc.vector.tensor_reduce(ssum[:], ex[:], axis=X, op=OP.add)
                rinv = rpool.tile([P, 1], f32, tag="rinv")
                nc.vector.reciprocal(rinv[:], ssum[:])
                smx = rpool.tile([P, E], f32, tag="smx")
                nc.vector.tensor_scalar(out=smx[:], in0=ex[:], scalar1=rinv[:, 0:1],
                                        scalar2=None, op0=OP.mult)
                nc.sync.dma_start(sm_d[t * P:(t + 1) * P, :], smx[:])
                # top-2 dispatch mask: g1 at argmax1, g2 at argmax2
                is1 = rpool.tile([P, E], f32, tag="is1")
                nc.vector.tensor_scalar(out=is1[:], in0=lgs[:], scalar1=m1[:, 0:1],
                                        scalar2=None, op0=OP.is_equal)
                mskd = rpool.tile([P, E], f32, tag="mskd")
                nc.vector.scalar_tensor_tensor(out=mskd[:], in0=is1[:], scalar=-1e30,
                                               in1=lgs[:], op0=OP.mult, op1=OP.add)
                m2 = rpool.tile([P, 1], f32, tag="m2")
                nc.vector.tensor_reduce(m2[:], mskd[:], axis=X, op=OP.max)
                is2 = rpool.tile([P, E], f32, tag="is2")
                nc.vector.tensor_scalar(out=is2[:], in0=mskd[:], scalar1=m2[:, 0:1],
                                        scalar2=None, op0=OP.is_equal)
                # g1 = sigmoid(m1-m2) as 1/(1+exp(m2-m1)): ACT only needs Exp
                d12 = rpool.tile([P, 1], f32, tag="d12")
                nc.vector.tensor_tensor(out=d12[:], in0=m2[:], in1=m1[:],
                                        op=OP.subtract)
                e12 = rpool.tile([P, 1], f32, tag="e12")
                nc.scalar.activation(e12[:], d12[:], AF.Exp)
                nc.vector.tensor_scalar(out=e12[:], in0=e12[:], scalar1=1.0,
                                        scalar2=None, op0=OP.add)
                g1 = rpool.tile([P, 1], f32, tag="g1")
                nc.vector.reciprocal(g1[:], e12[:])
                g2 = rpool.tile([P, 1], f32, tag="g2")
                nc.vector.tensor_scalar(out=g2[:], in0=g1[:], scalar1=-1.0,
                                        scalar2=1.0, op0=OP.mult, op1=OP.add)
                mk1 = rpool.tile([P, E], f32, tag="mk1")
                nc.vector.tensor_scalar(out=mk1[:], in0=is1[:], scalar1=g1[:, 0:1],
                                        scalar2=None, op0=OP.mult)
                mks = rpool.tile([P, E], f32, tag="mks")
                nc.vector.scalar_tensor_tensor(out=mks[:], in0=is2[:],
                                               scalar=g2[:, 0:1], in1=mk1[:],
                                               op0=OP.mult, op1=OP.add)
                nc.sync.dma_start(mk_d[t * P:(t + 1) * P, :], mks[:])

            def mm1_half(c, half, xc, halves, w1_sb, b1_sb):
                # computes hT half-tile (12 of 24 k-blocks) for chunk c
                ht = hpool.tile([P, KF // 2, CHUNK], mdt, tag="hT")
                halves[(c, half)] = ht
                for mi in range(KF // 2):
                    m = half * (KF // 2) + mi
                    hp = ps_h.tile([P, CHUNK], f32, tag="hp")
                    for k in range(KH):
                        nc.tensor.matmul(
                            hp[:], w1_sb[:, k, m * P:(m + 1) * P], xc[:, k, :],
                            start=(k == 0), stop=(k == KH - 1),
                        )
                    if gelu_mode == "lut":
                        nc.scalar.activation(ht[:, mi, :], hp[:],
                                             AF.Gelu_apprx_tanh,
                                             bias=b1_sb[:, m:m + 1])
                    else:
                        # explicit tanh gelu (CoreSim-friendly)
                        xb = ypool.tile([P, CHUNK], f32, tag="xb")
                        nc.scalar.activation(xb[:], hp[:], AF.Identity,
                                             bias=b1_sb[:, m:m + 1])
                        sq = ypool.tile([P, CHUNK], f32, tag="sq")
                        nc.scalar.activation(sq[:], xb[:], AF.Square)
                        poly = ypool.tile([P, CHUNK], f32, tag="poly")
                        nc.vector.tensor_scalar(out=poly[:], in0=sq[:], scalar1=C1,
                                                scalar2=C0, op0=OP.mult, op1=OP.add)
                        nc.vector.tensor_tensor(out=poly[:], in0=poly[:], in1=xb[:],
                                                op=OP.mult)
                        th = ypool.tile([P, CHUNK], f32, tag="th")
                        nc.scalar.activation(th[:], poly[:], AF.Tanh)
                        nc.vector.tensor_scalar(out=th[:], in0=th[:], scalar1=1.0,
                                                scalar2=0.5, op0=OP.add, op1=OP.mult)
                        nc.vector.tensor_tensor(out=ht[:, mi, :], in0=th[:],
                                                in1=xb[:], op=OP.mult)

            def mm2_chunk(c, halves, w2_sb, b2_sb, g_sb):
                hTa = halves.pop((c, 0))
                hTb = halves.pop((c, 1))
                for tt in range(CHUNK // P):
                    yp = ps_y.tile([P, H], f32, tag="yp")
                    for k in range(KF):
                        src_t = hTa if k < KF // 2 else hTb
                        hTk = src_t[:, k % (KF // 2), tt * P:(tt + 1) * P]
                        nc.tensor.matmul(yp[:, 0:512], hTk,
                                         w2_sb[:, k, 0:512],
                                         start=(k == 0), stop=(k == KF - 1))
                        nc.tensor.matmul(yp[:, 512:H], hTk,
                                         w2_sb[:, k, 512:H],
                                         start=(k == 0), stop=(k == KF - 1))
                    ys = ypool.tile([P, H], f32, tag="ys")
                    j = c * (CHUNK // P) + tt
                    if b2_zero:
                        nc.vector.tensor_scalar(out=ys[:], in0=yp[:],
                                                scalar1=g_sb[:, j:j + 1],
                                                scalar2=None, op0=OP.mult)
                    else:
                        nc.vector.tensor_tensor(out=ys[:], in0=yp[:],
                                                in1=b2_sb[:], op=OP.add)
                        nc.vector.tensor_scalar(out=ys[:], in0=ys[:],
                                                scalar1=g_sb[:, j:j + 1],
                                                scalar2=None, op0=OP.mult)
                    nc.sync.dma_start(y_d[j * P:(j + 1) * P, :], ys[:])

            for _rep in range(reps):
                nts = ntr if "router" in sections else 0
                ncs = nch if "ffn" in sections else 0

                # --- DMA priority order: tiny consts + first working set
                # first, then the big weight streams (in the order the PE
                # consumes them), so the PE starts within ~10us instead of
                # waiting for the full 19MB weight load.
                xr0 = load_xr(0) if nts else None
                rwt_sb = cpool.tile([P, KH, E], f32, tag="rwt")
                nc.sync.dma_start(rwt_sb[:], rwt_d[:])
                rb_sb = cpool.tile([P, E], f32, tag="rb")
                nc.sync.dma_start(rb_sb[:], rb_d[:])
                xc0 = load_xc(0) if ncs else None

                # w1 streamed in column blocks (mm1 consumes columns in
                # order); w2 streamed in k blocks (mm2 consumption order).
                # b1/gates are not needed until the first gelu/eviction, so
                # they queue after the first w1 block.
                w1_sb = wpool.tile([P, KH, F], mdt, tag="w1")
                nc.sync.dma_start(w1_sb[:, :, 0:128], w1_d[:, :, 0:128])
                b1_sb = cpool.tile([P, KF], f32, tag="b1")
                nc.sync.dma_start(b1_sb[:], b1_d[:])
                b2_sb = None
                if not b2_zero:
                    b2_sb = cpool.tile([P, H], f32, tag="b2")
                    nc.sync.dma_start(b2_sb[:], b2_d[:])
                g_sb = cpool.tile([P, C // P], f32, tag="g")
                nc.sync.dma_start(g_sb[:], g_d[:])
                nc.sync.dma_start(w1_sb[:, :, 128:384], w1_d[:, :, 128:384])
                for ms in range(384, F, 384):
                    nc.sync.dma_start(w1_sb[:, :, ms:ms + 384],
                                      w1_d[:, :, ms:ms + 384])
                w2_sb = wpool.tile([P, KF, H], mdt, tag="w2")
                for k in range(0, KF, 2):
                    nc.sync.dma_start(w2_sb[:, k:k + 2, :], w2_d[:, k:k + 2, :])

                # software pipeline: mm1 halves of chunk c+1/c+2 are emitted
                # around mm2(c) so the PE has mm1 work to run while mm2
                # trails the streaming w2 during startup. Router tiles are
                # interleaved one per chunk so their vector/scalar chains
                # hide under the FFN matmuls.
                halves = {}
                xcs = {0: xc0}
                if ncs:
                    if nts:
                        router_tile(0, rwt_sb, rb_sb, xr_sb=xr0)
                    mm1_half(0, 0, xcs[0], halves, w1_sb, b1_sb)
                    mm1_half(0, 1, xcs[0], halves, w1_sb, b1_sb)
                    if ncs > 1:
                        xcs[1] = load_xc(1)
                        mm1_half(1, 0, xcs[1], halves, w1_sb, b1_sb)
                    for c in range(ncs):
                        if 1 + c < nts:
                            router_tile(1 + c, rwt_sb, rb_sb)
                        mm2_chunk(c, halves, w2_sb, b2_sb, g_sb)
                        if c + 1 < ncs:
                            mm1_half(c + 1, 1, xcs[c + 1], halves, w1_sb, b1_sb)
                        if c + 2 < ncs:
                            xcs[c + 2] = load_xc(c + 2)
                            mm1_half(c + 2, 0, xcs[c + 2], halves, w1_sb, b1_sb)
                        xcs.pop(c, None)
                    for t in range(ncs + 1, nts):
                        router_tile(t, rwt_sb, rb_sb)
                else:
                    for t in range(nts):
                        router_tile(t, rwt_sb, rb_sb, xr_sb=xr0 if t == 0 else None)

    nc.compile()
    return nc


def _xT_blocks(xmat, kblocks):
    """[N, kblocks*P] row-major -> [P, kblocks, N] (transposed block layout)."""
    n = xmat.shape[0]
    return np.ascontiguousarray(
        xmat.T.reshape(kblocks, P, n).transpose(1, 0, 2)
    )


def host_route(x, router_w, router_b):
    """numpy replica of the routing decision (indices + gates for dispatch)."""
    logits = (x @ router_w + router_b).astype(np.float32)
    idx1 = np.argmax(logits, axis=1)
    rows = np.arange(x.shape[0])
    masked = logits.copy()
    masked[rows, idx1] = -np.inf
    idx2 = np.argmax(masked, axis=1)
    v1 = logits[rows, idx1]
    v2 = logits[rows, idx2]
    g1 = (1.0 / (1.0 + np.exp((v2 - v1).astype(np.float64)))).astype(np.float32)
    g2 = np.float32(1.0) - g1
    return logits, idx1, idx2, g1, g2


def kernel(hidden_states, router_w, router_b, w1, b1, w2, b2):
    hidden_states = np.asarray(hidden_states, dtype=np.float32)
    router_w = np.asarray(router_w, dtype=np.float32)
    router_b = np.asarray(router_b, dtype=np.float32)
    w1 = np.asarray(w1, dtype=np.float32)
    b1 = np.asarray(b1, dtype=np.float32)
    w2 = np.asarray(w2, dtype=np.float32)
    b2 = np.asarray(b2, dtype=np.float32)

    try:
        import jax
        jax.config.update("jax_compilation_cache_dir", "/tmp/jax_cache")
        jax.config.update("jax_persistent_cache_min_compile_time_secs", 1.0)
    except Exception:
        pass

    B, S, Hd = hidden_states.shape
    assert Hd == H
    T = B * S
    assert T % N_CORES == 0
    TS = T // N_CORES
    x = np.ascontiguousarray(hidden_states.reshape(T, H))

    # ---- host routing decision (for the expert-parallel gather only) ----
    logits_h, idx1, idx2, g1, g2 = host_route(x, router_w, router_b)

    tok_lists, gate_lists = [], []
    for e in range(E):
        s1 = idx1 == e
        s2 = idx2 == e
        tok = np.nonzero(s1 | s2)[0]
        gate = np.where(s1[tok], g1[tok], g2[tok]).astype(np.float32)
        tok_lists.append(tok)
        gate_lists.append(gate)
    maxcnt = max(len(t) for t in tok_lists)
    C = int(-(-maxcnt // CHUNK) * CHUNK)

    reps = int(os.environ.get("MOE_REPS", "1"))
    mm_dtype = os.environ.get("MOE_MM_DTYPE", "float32r")
    gelu_mode = os.environ.get("MOE_GELU", "lut")
    b2_zero = bool(np.all(b2 == 0))
    key = (C, TS, reps, mm_dtype, gelu_mode, b2_zero)
    if key not in _PROGRAM_CACHE:
        _PROGRAM_CACHE[key] = build_program(
            C, TS, reps=reps, mm_dtype_name=mm_dtype, gelu_mode=gelu_mode,
            b2_zero=b2_zero)
    nc = _PROGRAM_CACHE[key]

    import ml_dtypes
    np_mdt = np.float32 if mm_dtype == "float32r" else ml_dtypes.bfloat16

    rwt_arr = np.ascontiguousarray(
        router_w.reshape(KH, P, E).transpose(1, 0, 2))
    rb_arr = np.ascontiguousarray(np.broadcast_to(router_b, (P, E)))

    in_maps = []
    for e in range(E):
        tok = tok_lists[e]
        n = len(tok)
        xg = np.zeros((C, H), dtype=np.float32)
        xg[:n] = x[tok]
        gates = np.zeros((C,), dtype=np.float32)
        gates[:n] = gate_lists[e]
        xs = x[e * TS:(e + 1) * TS]
        im = {
            "xT_ffn": _xT_blocks(xg, KH).astype(np_mdt),
            "w1": np.ascontiguousarray(
                w1[e].reshape(KH, P, F).transpose(1, 0, 2)).astype(np_mdt),
            "w2": np.ascontiguousarray(
                w2[e].reshape(KF, P, H).transpose(1, 0, 2)).astype(np_mdt),
            "b1": np.ascontiguousarray(b1[e].reshape(KF, P).T),
            "gates": np.ascontiguousarray(gates.reshape(C // P, P).T),
            "x_r": _xT_blocks(xs, KH),
            "rwght": rwt_arr,
            "rbias": rb_arr,
        }
        if not b2_zero:
            im["b2"] = np.ascontiguousarray(np.broadcast_to(b2[e], (P, H)))
        in_maps.append(im)

    from concourse.bass_utils import run_bass_kernel_spmd
    res = run_bass_kernel_spmd(nc, in_maps, core_ids=list(range(N_CORES)))
    results = res.results

    # ---- gather / unshard ----
    combined = np.zeros((T, H), dtype=np.float32)
    for e in range(E):
        tok = tok_lists[e]
        combined[tok] += results[e]["y"][:len(tok)]

    logits = np.concatenate([results[i]["logits"] for i in range(N_CORES)], axis=0)
    softmax = np.concatenate([results[i]["softmax"] for i in range(N_CORES)], axis=0)
    mask = np.concatenate([results[i]["mask"] for i in range(N_CORES)], axis=0)
    mean_prob = softmax.mean(axis=0, dtype=np.float64)
    tokens_per_expert = mask.sum(axis=0, dtype=np.float64) / T
    aux = np.float32(E * np.dot(mean_prob, tokens_per_expert))

    return (
        combined.reshape(B, S, H),
        softmax.reshape(B, S, E),
        mask.reshape(B, S, E),
        np.float32(aux),
        logits.reshape(B, S, E),
    )
